# revision 6
# baseline (speedup 1.0000x reference)
"""Mix9Net forward pass on 8 Trainium2 NeuronCores (Bass/Tile).

Data parallel over batch (256 -> 32 per core). The conv trunk (4 directional
silu-resnets) runs in fp16 storage with fp32 PSUM accumulation; matmuls read
interior pixels through strided APs over zero-ring padded per-sample layouts.
The quantized heads run in exact integer arithmetic (values held at x128
integer scale); fake_quant's round() is the fp32 magic-number RNE trick and
floor() is RNE plus an is_gt fixup. Weight preprocessing (quantization,
transposes, scale folding) happens on host with jax-identical semantics.
"""
import sys

sys.path.insert(0, "/opt/trn_rl_repo")

import numpy as np

import concourse.bass as bass
import concourse.tile as tile
from concourse import mybir
from concourse.bass_utils import run_bass_kernel_spmd
from concourse.vector_clock import ScopedClock, VectorClock

F32 = mybir.dt.float32
F16 = mybir.dt.float16
AF = mybir.ActivationFunctionType
ALU = mybir.AluOpType
AX = mybir.AxisListType

MAGIC = float(np.float32(1.5 * 2.0**23))

N_CORES = 8
BL = 32                   # samples per core
S = 289                   # 17*17 padded sample
SLK = 18                  # head slack
PADW = SLK + BL * S + 20  # padded-flat width per partition
HW = 225

DIR_POS = (((1, 0), (1, 1), (1, 2)),
           ((0, 1), (1, 1), (2, 1)),
           ((0, 0), (1, 1), (2, 2)),
           ((2, 0), (1, 1), (0, 2)))

# ---------------------------------------------------------------- walrus glue


class TC(tile.TileContext):
    """This container's walrus accepts at most ONE sync wait per instruction;
    emit the kernel-tail drain as a chain of single-wait drains."""

    def _drain_and_barrier(self, tick_clock, wait_clock):
        gc = tick_clock.global_clock
        n = len(gc)
        for i in range(n):
            if gc[i] <= 0:
                continue
            vec = [0] * n
            vec[i] = gc[i]
            d = self.nc.sync.drain()
            wait_clock.add_sem_waits(d.ins, ScopedClock({None: VectorClock(vec)}))
        self.nc.all_engine_barrier()
        popped = self.nc._tile_sem_poison_stack.pop()
        assert popped is self._sem_poison
        self.nc.clear_and_free_semaphores(list(self.sems.allocated().values()))
        self.nc.all_engine_barrier()


def split_waits(nc):
    """Move excess sync waits onto same-engine NOPs placed just before the
    overloaded instruction (program order preserved -> semantics unchanged)."""
    for f in nc.m.functions:
        for bb in f.blocks:
            out = []
            for inst in bb.instructions:
                si = inst.sync_info
                if si is not None:
                    budget = max(0, min(1, 2 - len(si.on_update)))
                    if len(si.on_wait) > budget:
                        waits = list(si.on_wait)
                        nkeep = len(waits) - budget
                        extra, keep = waits[:nkeep], waits[nkeep:]
                        for i, w in enumerate(extra):
                            nop = mybir.InstNoOp(name=f"{inst.name}-wsp{i}",
                                                 engine=inst.engine)
                            nop.sync_info = mybir.SyncInfo(on_wait=[w], on_update=[])
                            out.append(nop)
                        inst.sync_info = mybir.SyncInfo(on_wait=list(keep),
                                                        on_update=list(si.on_update))
                out.append(inst)
            bb.instructions[:] = out


# ---------------------------------------------------------------- host prep

def _fq(x, scale, bits, floor=False):
    x = np.asarray(x, np.float32)
    qmin = np.float32(-(2.0 ** (bits - 1)))
    qmax = np.float32(2.0 ** (bits - 1) - 1)
    y = np.clip(x * np.float32(scale), qmin, qmax)
    q = np.floor(y) if floor else np.round(y)
    return (q.astype(np.float32) / np.float32(scale)).astype(np.float32)


def prep_weights(params):
    out = {}
    tmap = {}
    tcols = []

    def addt(name, a):
        tmap[name] = (sum(x.shape[1] for x in tcols), a.shape[1])
        tcols.append(np.ascontiguousarray(np.asarray(a, np.float32)))

    for mi, mkey in enumerate(("map1", "map2")):
        p = params[mkey]
        for bi, blk in enumerate(p["blocks"]):
            for k in range(3):
                addt(f"dcw{mi}_{bi}_{k}", np.asarray(blk["dcw"], np.float32)[k].T)
            addt(f"cw{mi}_{bi}", np.asarray(blk["cw"], np.float32).T)
        addt(f"c01_{mi}", np.asarray(p["c0_1w"], np.float32).T)
        addt(f"c02_{mi}", np.asarray(p["c0_2w"], np.float32).T)
        addt(f"fw_{mi}", np.asarray(p["fw"], np.float32).T)
    ncol = sum(a.shape[1] for a in tcols)
    wtrunk = np.zeros((128, ncol), np.float16)
    c = 0
    for a in tcols:
        wtrunk[: a.shape[0], c:c + a.shape[1]] = a.astype(np.float16)
        c += a.shape[1]
    out["wtrunk"] = wtrunk

    w18 = np.zeros((18, 4 * 128), np.float32)
    for d in range(4):
        mkey = "map1" if d < 2 else "map2"
        dw = np.asarray(params[mkey]["dw"], np.float32)  # [3, 128, 2]
        for k, (i, j) in enumerate(DIR_POS[d]):
            s = ((i - 1) + 1) * 3 + ((j - 1) + 1)
            for cc in range(2):
                w18[s * 2 + cc, d * 128:(d + 1) * 128] += dw[k, :, cc]
    out["w18t"] = w18.astype(np.float16)

    bmap = {}
    bt = np.zeros((128, 24), np.float32)
    bc = [0]

    def addb(name, v):
        bmap[name] = bc[0]
        v = np.asarray(v, np.float32)
        bt[: len(v), bc[0]] = v
        bc[0] += 1

    for mi, mkey in enumerate(("map1", "map2")):
        p = params[mkey]
        addb(f"db{mi}", p["db"])
        for bi, blk in enumerate(p["blocks"]):
            addb(f"dcb{mi}_{bi}", blk["dcb"])
            addb(f"cb{mi}_{bi}", blk["cb"])
        addb(f"c01b{mi}", p["c0_1b"])
        addb(f"c02b{mi}", p["c0_2b"])
        addb(f"fb{mi}", p["fb"])
    out["btrunk"] = bt

    wq = _fq(params["dwconv_w"], 65536, 16)
    wdw = np.zeros((64, 288), np.float16)
    for i in range(3):
        for j in range(3):
            k = 3 * i + j
            wdw[32:64, 32 * k:32 * k + 32] = (np.diag(wq[:, 0, i, j]) / 32.0
                                              ).astype(np.float16)
    out["wdw"] = wdw

    hmap = {}
    hcols = []

    def addh(name, a):
        hmap[name] = (sum(x.shape[1] for x in hcols), a.shape[1])
        hcols.append(np.asarray(a, np.float32))

    addh("pw1T", _fq(params["pw1_w"], 128, 8).T)
    addh("pw2T", _fq(params["pw2_w"], 128, 8).T)
    for sk in ("vc", "ve", "vm", "vq"):
        sp = params[sk]
        addh(f"{sk}_u1T", _fq(sp["u1w"], 128, 8).T)
        addh(f"{sk}_u2T", _fq(sp["u2w"], 128, 8).T)
        addh(f"{sk}_dwT", _fq(sp["dw"], 128, 8).T)
    vl1 = _fq(params["vl1_w"], 128, 8).T
    addh("vl1T0", vl1[0:128])
    addh("vl1T1", vl1[128:256])
    addh("vl1T2", vl1[256:320])
    addh("vl2T", _fq(params["vl2_w"], 128, 8).T)
    addh("vl3T", _fq(params["vl3_w"], 128, 8).T)
    ncol = sum(a.shape[1] for a in hcols)
    whead = np.zeros((128, ncol), np.float16)
    c = 0
    for a in hcols:
        whead[: a.shape[0], c:c + a.shape[1]] = a.astype(np.float16)
        c += a.shape[1]
    out["whead"] = whead

    whf = np.zeros((128, 65), np.float32)
    for j in range(64):
        whf[2 * j, j] = 1.0
        whf[2 * j + 1, j] = 1.0
    whf[0:16, 64] = np.asarray(params["pout_w"], np.float32)[0]
    out["wheadf"] = whf

    bhmap = {}
    bh = np.zeros((128, 24), np.float32)
    hc = [0]

    def addbh(name, v, dup64=False):
        bhmap[name] = hc[0]
        v = np.asarray(v, np.float32).reshape(-1)
        bh[: len(v), hc[0]] = v
        if dup64:
            bh[64:64 + len(v), hc[0]] = v
        hc[0] += 1

    for sk in ("vc", "ve", "vm", "vq"):
        sp = params[sk]
        addbh(f"{sk}_u1b", _fq(sp["u1b"], 128 * 128, 32) * 128)
        addbh(f"{sk}_u2b", _fq(sp["u2b"], 128 * 128, 32) * 128)
        addbh(f"{sk}_dwb", _fq(sp["db"], 128 * 128, 32) * 128, dup64=True)
    addbh("pw1b", _fq(params["pw1_b"], 128 * 128, 32) * 128)
    addbh("vl1b", _fq(params["vl1_b"], 128 * 128, 32) * 128)
    addbh("vl2b", _fq(params["vl2_b"], 128 * 128, 32) * 128)
    addbh("vl3b", _fq(params["vl3_b"], 128 * 128, 32))
    addbh("poutb", params["pout_b"])
    addbh("dwb", _fq(params["dwconv_b"], 128, 16) * 128)
    pw2b = _fq(params["pw2_b"], 128 * 128, 32) * 128
    for j in range(5):
        addbh(f"pw2b{j}", pw2b[128 * j:min(528, 128 * (j + 1))])
    out["bhead"] = bh

    return out, tmap, bmap, hmap, bhmap


# ---------------------------------------------------------------- device build

_prog_cache = {}


def build_program(tmap, bmap, hmap, bhmap, shapes, debug=False):
    nc = bass.Bass()
    dram = {}

    def din(name, shape, dt):
        dram[name] = nc.dram_tensor(name, list(shape), dt, kind="ExternalInput")
        return dram[name]

    def dout(name, shape, dt=F32):
        dram[name] = nc.dram_tensor(name, list(shape), dt, kind="ExternalOutput")
        return dram[name]

    board_d = din("board", [2, BL * S], F16)  # host-padded 17x17
    w18_d = din("w18t", shapes["w18t"], F16)
    wtrunk_d = din("wtrunk", shapes["wtrunk"], F16)
    btrunk_d = din("btrunk", shapes["btrunk"], F32)
    wdw_d = din("wdw", shapes["wdw"], F16)
    whead_d = din("whead", shapes["whead"], F16)
    wheadf_d = din("wheadf", shapes["wheadf"], F32)
    bhead_d = din("bhead", shapes["bhead"], F32)
    value_d = dout("value", [3, BL])
    policy_d = dout("policy", [BL, HW])
    if debug:
        dout("facc_dbg", [64, 7200])
        dout("fdq_dbg", [32, 7200])
        dout("fsum_dbg", [64, BL])
        dout("pwc_dbg", [32, 16 * BL])
        dout("xa_dbg", [128, PADW])

    with TC(nc) as tc:
        with tc.tile_pool(name="wpool", bufs=1) as wpool:
            # ---- persistent tiles
            xA = wpool.tile([128, PADW], F16)
            xB = wpool.tile([128, PADW], F16)
            X18 = wpool.tile([18, PADW], F16)
            PAD1 = wpool.tile([128, PADW], F16)  # r0-1 board, r32-63 fdin, r64-127 frel
            facc = wpool.tile([64, 7200], F32)
            fdq = wpool.tile([32, 7200], F32)
            wtrunk = wpool.tile([128, shapes["wtrunk"][1]], F16)
            w18t = wpool.tile([18, shapes["w18t"][1]], F16)
            btrunk = wpool.tile([128, shapes["btrunk"][1]], F32)
            wdw = wpool.tile([64, shapes["wdw"][1]], F16)
            whead = wpool.tile([128, shapes["whead"][1]], F16)
            wheadf = wpool.tile([128, shapes["wheadf"][1]], F32)
            bhead = wpool.tile([128, shapes["bhead"][1]], F32)

            nc.vector.memset(xA[:], 0.0)
            nc.gpsimd.memset(xB[:], 0.0)
            nc.vector.memset(PAD1[:], 0.0)

            nc.gpsimd.dma_start(out=wtrunk[:], in_=wtrunk_d[:])
            nc.gpsimd.dma_start(out=w18t[:], in_=w18_d[:])
            nc.gpsimd.dma_start(out=btrunk[:], in_=btrunk_d[:])
            nc.gpsimd.dma_start(out=wdw[:], in_=wdw_d[:])
            nc.gpsimd.dma_start(out=whead[:], in_=whead_d[:])
            nc.gpsimd.dma_start(out=wheadf[:], in_=wheadf_d[:])
            nc.gpsimd.dma_start(out=bhead[:], in_=bhead_d[:])

            def padview(t):  # [128, BL, 17, 17]
                return t[:, SLK:SLK + BL * S].rearrange(
                    "p (b h w) -> p b h w", h=17, w=17)

            def interior(t, b0, nb):  # [*, nb, 15, 15]
                return padview(t)[:, b0:b0 + nb, 1:16, 1:16]

            def interior5(t, g):  # [*, 4, 2, 15, 15] for group g (8 samples)
                v = t[:, SLK + 8 * g * S: SLK + (8 * g + 8) * S].rearrange(
                    "p (a b h w) -> p a b h w", a=4, b=2, h=17, w=17)
                return v[:, :, :, 1:16, 1:16]

            pv1 = padview(PAD1)
            nc.gpsimd.dma_start(out=PAD1[0:2, SLK:SLK + BL * S], in_=board_d[:])

            for s in range(9):
                di, dj = s // 3 - 1, s % 3 - 1
                sh = di * 17 + dj
                nc.sync.dma_start(
                    out=X18[2 * s:2 * s + 2, SLK:SLK + BL * S],
                    in_=PAD1[0:2, SLK + sh:SLK + sh + BL * S],
                )
            x18v = padview(X18)

            def tb(name):
                return btrunk[:, bmap[name]:bmap[name] + 1]

            def wcol(name):
                c0, w = tmap[name]
                return wtrunk[:, c0:c0 + w]

            with tc.tile_pool(name="ppool", bufs=2, space="PSUM") as ppool:
                # ================= trunk =================
                for d in range(4):
                    mi = d // 2
                    shifts = [((i - 1), (j - 1)) for (i, j) in DIR_POS[d]]
                    cur, nxt = (xA, xB)

                    for g in range(4):
                        ps = ppool.tile([128, 4, 512], F32, name="ps1", tag="ps")
                        for pr in range(4):
                            b0 = 8 * g + 2 * pr
                            nc.tensor.matmul(
                                ps[:, pr, 0:450],
                                w18t[:, d * 128:(d + 1) * 128],
                                x18v[:, b0:b0 + 2, 1:16, 1:16],
                                start=True, stop=True,
                            )
                        nc.scalar.activation(
                            interior5(cur, g),
                            ps[:, :, 0:450].rearrange(
                                "p a (b h w) -> p a b h w", b=2, h=15, w=15),
                            AF.Silu, bias=tb(f"db{mi}"), scale=1.0,
                        )

                    for bi in range(4):
                        for g in range(4):
                            ps = ppool.tile([128, 4, 512], F32, name="psd", tag="ps")
                            for pr in range(4):
                                b0 = 8 * g + 2 * pr
                                for k in range(3):
                                    di, dj = shifts[k]
                                    rhs = padview(cur)[:, b0:b0 + 2,
                                                       1 + di:16 + di, 1 + dj:16 + dj]
                                    nc.tensor.matmul(
                                        ps[:, pr, 0:450],
                                        wcol(f"dcw{mi}_{bi}_{k}"),
                                        rhs,
                                        start=(k == 0), stop=(k == 2),
                                    )
                            tg = wpool.tile([128, 1800], F16, name="tg",
                                            tag="tg", bufs=2)
                            nc.scalar.activation(
                                tg[:].rearrange("p (a b) -> p a b", a=4),
                                ps[:, :, 0:450],
                                AF.Silu, bias=tb(f"dcb{mi}_{bi}"), scale=1.0,
                            )
                            ps2 = ppool.tile([128, 4, 512], F32, name="psc", tag="ps")
                            for pr in range(4):
                                nc.tensor.matmul(
                                    ps2[:, pr, 0:450],
                                    wcol(f"cw{mi}_{bi}"),
                                    tg[:, 450 * pr:450 * (pr + 1)],
                                    start=True, stop=True,
                                )
                            tsil = wpool.tile([128, 1800], F16, name="tsil",
                                              tag="tsil", bufs=3)
                            nc.scalar.activation(
                                tsil[:].rearrange("p (a b) -> p a b", a=4),
                                ps2[:, :, 0:450],
                                AF.Silu, bias=tb(f"cb{mi}_{bi}"), scale=1.0,
                            )
                            eng = nc.vector if (bi % 2 == 0) else nc.gpsimd
                            eng.tensor_tensor(
                                out=interior(nxt, 8 * g, 8),
                                in0=tsil[:].rearrange("p (b h w) -> p b h w",
                                                      h=15, w=15),
                                in1=interior(cur, 8 * g, 8),
                                op=ALU.add,
                            )
                        cur, nxt = nxt, cur

                    for g in range(4):
                        ps = ppool.tile([128, 4, 512], F32, name="psu", tag="ps")
                        for pr in range(4):
                            b0 = 8 * g + 2 * pr
                            nc.tensor.matmul(
                                ps[:, pr, 0:450], wcol(f"c01_{mi}"),
                                interior(cur, b0, 2), start=True, stop=True,
                            )
                        u1 = wpool.tile([128, 1800], F16, name="u1", tag="tg", bufs=2)
                        nc.scalar.activation(
                            u1[:].rearrange("p (a b) -> p a b", a=4),
                            ps[:, :, 0:450],
                            AF.Silu, bias=tb(f"c01b{mi}"), scale=1.0,
                        )
                        ps2 = ppool.tile([128, 4, 512], F32, name="psu2", tag="ps")
                        for pr in range(4):
                            nc.tensor.matmul(
                                ps2[:, pr, 0:450], wcol(f"c02_{mi}"),
                                u1[:, 450 * pr:450 * (pr + 1)], start=True, stop=True,
                            )
                        tsil2 = wpool.tile([128, 1800], F16, name="tsil2",
                                           tag="tsil", bufs=3)
                        nc.scalar.activation(
                            tsil2[:].rearrange("p (a b) -> p a b", a=4),
                            ps2[:, :, 0:450],
                            AF.Silu, bias=tb(f"c02b{mi}"), scale=1.0,
                        )
                        x5 = wpool.tile([128, 1800], F16, name="x5",
                                        tag="tsil", bufs=3)
                        nc.gpsimd.tensor_tensor(
                            out=x5[:].rearrange("p (b h w) -> p b h w", h=15, w=15),
                            in0=tsil2[:].rearrange("p (b h w) -> p b h w", h=15, w=15),
                            in1=interior(cur, 8 * g, 8),
                            op=ALU.add,
                        )
                        psF = ppool.tile([64, 4, 512], F32, name="psf", tag="ps")
                        for pr in range(4):
                            nc.tensor.matmul(
                                psF[:, pr, 0:450], wcol(f"fw_{mi}"),
                                x5[:, 450 * pr:450 * (pr + 1)], start=True, stop=True,
                            )
                        z = wpool.tile([64, 1800], F32, name="z", tag="z", bufs=2)
                        zv = z[:].rearrange("p (a b) -> p a b", a=4)
                        nc.vector.tensor_scalar(
                            out=zv, in0=psF[:, :, 0:450],
                            scalar1=btrunk[0:64, bmap[f"fb{mi}"]:bmap[f"fb{mi}"] + 1],
                            scalar2=32.0, op0=ALU.add, op1=ALU.mult,
                        )
                        nc.gpsimd.tensor_scalar(
                            out=z[:], in0=z[:], scalar1=512.0, scalar2=-512.0,
                            op0=ALU.min, op1=ALU.max,
                        )
                        fslice = facc[:, 1800 * g:1800 * (g + 1)]
                        if d == 0:
                            nc.gpsimd.tensor_scalar(
                                out=fslice, in0=z[:], scalar1=MAGIC, scalar2=MAGIC,
                                op0=ALU.add, op1=ALU.subtract,
                            )
                        else:
                            nc.gpsimd.tensor_scalar(
                                out=z[:], in0=z[:], scalar1=MAGIC, scalar2=MAGIC,
                                op0=ALU.add, op1=ALU.subtract,
                            )
                            nc.vector.tensor_tensor(out=fslice, in0=z[:],
                                                    in1=fslice, op=ALU.add)

                # ============== feature stage ==============
                # frel (x128 ints, relu) -> PAD1 rows 64:128; fdin -> PAD1 rows 32:64
                nc.scalar.activation(PAD1[64:128, 0:7200], facc[:], AF.Relu,
                                     bias=0.0, scale=1.0)
                nc.scalar.activation(
                    pv1[32:64, :, 1:16, 1:16],
                    facc[0:32, :].rearrange("p (b h w) -> p b h w", h=15, w=15),
                    AF.Relu, bias=0.0, scale=1.0,
                )

                for g in range(4):
                    psdw = ppool.tile([32, 4, 512], F32, name="psdw", tag="ps")
                    for pr in range(4):
                        b0 = 8 * g + 2 * pr
                        for k in range(9):
                            di, dj = k // 3 - 1, k % 3 - 1
                            rhs = pv1[32:64, b0:b0 + 2,
                                      1 + di:16 + di, 1 + dj:16 + dj]
                            nc.tensor.matmul(
                                psdw[:, pr, 0:450],
                                wdw[32:64, 32 * k:32 * k + 32],
                                rhs,
                                start=(k == 0), stop=(k == 8),
                                tile_position=(32, 0),
                            )
                    fdt = wpool.tile([32, 1800], F32, name="fdt", tag="z", bufs=2)
                    nc.scalar.activation(
                        fdt[:].rearrange("p (a b) -> p a b", a=4),
                        psdw[:, :, 0:450], AF.Relu,
                        bias=bhead[0:32, bhmap["dwb"]:bhmap["dwb"] + 1], scale=128.0,
                    )
                    nc.gpsimd.tensor_scalar(
                        out=fdt[:], in0=fdt[:], scalar1=32767.0, scalar2=MAGIC,
                        op0=ALU.min, op1=ALU.add,
                    )
                    nc.gpsimd.tensor_scalar(
                        out=fdq[:, 1800 * g:1800 * (g + 1)], in0=fdt[:],
                        scalar1=MAGIC, scalar2=None, op0=ALU.subtract,
                    )

            if debug:
                nc.sync.dma_start(out=dram["facc_dbg"][:], in_=facc[:])
                nc.sync.dma_start(out=dram["fdq_dbg"][:], in_=fdq[:])
                nc.gpsimd.dma_start(out=dram["xa_dbg"][:], in_=xA[:])

            # ================= heads =================
            with (
                tc.tile_pool(name="hq", bufs=4) as hq,
                tc.tile_pool(name="hs", bufs=1) as hs,
                tc.tile_pool(name="hp", bufs=1, space="PSUM") as hp,
            ):
                frel = PAD1[64:128, 0:7200]

                fsumS = hs.tile([128, BL], F32)
                nc.vector.tensor_reduce(
                    out=fsumS[0:32, :],
                    in_=fdq[:].rearrange("p (b f) -> p b f", f=HW),
                    axis=AX.X, op=ALU.add,
                )
                nc.vector.tensor_reduce(
                    out=fsumS[96:128, :],
                    in_=frel[32:64, :].rearrange("p (b f) -> p b f", f=HW),
                    axis=AX.X, op=ALU.add,
                )
                nc.sync.dma_start(out=fsumS[32:64, :], in_=fsumS[96:128, :])

                def floor_int(src_ap, pre_mult, name):
                    """floor(src*pre_mult) over [64, BL] -> f32 ints."""
                    y = hq.tile([64, BL], F32, name=f"{name}_y", tag="fl_y")
                    nc.vector.tensor_scalar(out=y[:], in0=src_ap, scalar1=pre_mult,
                                            scalar2=None, op0=ALU.mult)
                    r = hq.tile([64, BL], F32, name=f"{name}_r", tag="fl_r")
                    nc.vector.tensor_scalar(out=r[:], in0=y[:], scalar1=MAGIC,
                                            scalar2=MAGIC, op0=ALU.add,
                                            op1=ALU.subtract)
                    gt = hq.tile([64, BL], F32, name=f"{name}_g", tag="fl_g")
                    nc.vector.tensor_tensor(out=gt[:], in0=r[:], in1=y[:],
                                            op=ALU.is_gt)
                    nc.vector.tensor_tensor(out=r[:], in0=r[:], in1=gt[:],
                                            op=ALU.subtract)
                    return r

                fsum_i = floor_int(fsumS[0:64, :], 1.0 / 256.0, "fsum")
                if debug:
                    nc.sync.dma_start(out=dram["fsum_dbg"][:], in_=fsum_i[:])

                pwin = hs.tile([64, BL], F16)
                nc.vector.tensor_scalar(out=pwin[:], in0=fsum_i[:], scalar1=127.0,
                                        scalar2=-128.0, op0=ALU.min, op1=ALU.max)

                hb = (0, 5, 10, 15)
                regq = {}
                for i in range(3):
                    for j in range(3):
                        rS = hq.tile([128, BL], F32, name=f"r{i}{j}", tag="regS")
                        nc.vector.tensor_reduce(
                            out=rS[0:32, :],
                            in_=fdq[:].rearrange("p (b h w) -> p b h w",
                                                 h=15, w=15)[
                                :, :, hb[i]:hb[i + 1], hb[j]:hb[j + 1]],
                            axis=AX.XY, op=ALU.add,
                        )
                        nc.vector.tensor_reduce(
                            out=rS[96:128, :],
                            in_=frel[32:64, :].rearrange("p (b h w) -> p b h w",
                                                         h=15, w=15)[
                                :, :, hb[i]:hb[i + 1], hb[j]:hb[j + 1]],
                            axis=AX.XY, op=ALU.add,
                        )
                        nc.sync.dma_start(out=rS[32:64, :], in_=rS[96:128, :])
                        ri = floor_int(rS[0:64, :], 1.0 / 32.0, f"ri{i}{j}")
                        q = hs.tile([64, BL], F16, name=f"regq{i}{j}",
                                    tag="regq", bufs=10)
                        nc.vector.tensor_scalar(out=q[:], in0=ri[:], scalar1=127.0,
                                                scalar2=-128.0, op0=ALU.min,
                                                op1=ALU.max)
                        regq[(i, j)] = q

                HROWS = {"pw1T": 64, "pw2T": 64, "vl1T0": 128, "vl1T1": 128,
                         "vl1T2": 64, "vl2T": 64, "vl3T": 64}

                def hcol(name):
                    c0, w = hmap[name]
                    rows = HROWS.get(name, 64)
                    return whead[0:rows, c0:c0 + w]

                def bh_(name, r0, rows):
                    return bhead[r0:r0 + rows, bhmap[name]:bhmap[name] + 1]

                def quant8_floor(v_ap, rows, relu, name):
                    c = hq.tile([128, BL], F32, name=f"{name}_c", tag="q8_c")
                    if relu:
                        nc.vector.tensor_scalar(out=c[0:rows], in0=v_ap,
                                                scalar1=127.0, scalar2=None,
                                                op0=ALU.min)
                    else:
                        nc.vector.tensor_scalar(out=c[0:rows], in0=v_ap,
                                                scalar1=127.0, scalar2=-128.0,
                                                op0=ALU.min, op1=ALU.max)
                    r = hq.tile([128, BL], F32, name=f"{name}_r", tag="q8_r")
                    nc.vector.tensor_scalar(out=r[0:rows], in0=c[0:rows],
                                            scalar1=MAGIC, scalar2=MAGIC,
                                            op0=ALU.add, op1=ALU.subtract)
                    g = hq.tile([128, BL], F32, name=f"{name}_g", tag="q8_g")
                    nc.vector.tensor_tensor(out=g[0:rows], in0=r[0:rows],
                                            in1=c[0:rows], op=ALU.is_gt)
                    q = hq.tile([128, BL], F16, name=f"{name}_q", tag="q8_q")
                    nc.vector.tensor_tensor(out=q[0:rows], in0=r[0:rows],
                                            in1=g[0:rows], op=ALU.subtract)
                    return q

                def star(xq, pset, col_off, name):
                    ps1 = hp.tile([128, BL], F32, name=f"{name}_p1", tag="st_p1")
                    nc.tensor.matmul(ps1[:], hcol(f"{pset}_u1T"), xq[0:64],
                                     start=True, stop=True)
                    v1 = hq.tile([128, BL], F32, name=f"{name}_v1", tag="st_v1")
                    nc.scalar.activation(v1[:], ps1[:], AF.Relu,
                                         bias=bh_(f"{pset}_u1b", 0, 128), scale=1.0)
                    x1q = quant8_floor(v1[:], 128, True, f"{name}_x1")
                    ps2 = hp.tile([128, BL], F32, name=f"{name}_p2", tag="st_p2")
                    nc.tensor.matmul(ps2[:], hcol(f"{pset}_u2T"), xq[0:64],
                                     start=True, stop=True)
                    v2 = hq.tile([128, BL], F32, name=f"{name}_v2", tag="st_v2")
                    nc.scalar.activation(v2[:], ps2[:], AF.Identity,
                                         bias=bh_(f"{pset}_u2b", 0, 128), scale=1.0)
                    x2q = quant8_floor(v2[:], 128, False, f"{name}_x2")
                    p = hq.tile([128, BL], F32, name=f"{name}_pp", tag="st_pp")
                    nc.vector.tensor_tensor(out=p[:], in0=x1q[:], in1=x2q[:],
                                            op=ALU.mult)
                    ps3 = hp.tile([64, BL], F32, name=f"{name}_p3", tag="st_p3")
                    nc.tensor.matmul(ps3[:], wheadf[:, 0:64], p[:],
                                     start=True, stop=True)
                    y3 = hq.tile([64, BL], F32, name=f"{name}_y3", tag="st_y3")
                    nc.vector.tensor_scalar(out=y3[:], in0=ps3[:],
                                            scalar1=1.0 / 128.0, scalar2=127.0,
                                            op0=ALU.mult, op1=ALU.min)
                    nc.vector.tensor_scalar(out=y3[:], in0=y3[:], scalar1=-128.0,
                                            scalar2=None, op0=ALU.max)
                    r3 = hq.tile([64, BL], F32, name=f"{name}_r3", tag="st_r3")
                    nc.vector.tensor_scalar(out=r3[:], in0=y3[:], scalar1=MAGIC,
                                            scalar2=MAGIC, op0=ALU.add,
                                            op1=ALU.subtract)
                    g3 = hq.tile([64, BL], F32, name=f"{name}_g3", tag="st_g3")
                    nc.vector.tensor_tensor(out=g3[:], in0=r3[:], in1=y3[:],
                                            op=ALU.is_gt)
                    x3q = hq.tile([64, BL], F16, name=f"{name}_x3", tag="st_x3")
                    nc.vector.tensor_tensor(out=x3q[:], in0=r3[:], in1=g3[:],
                                            op=ALU.subtract)
                    ps4 = hp.tile([128, BL], F32, name=f"{name}_p4", tag="st_p4")
                    sl = slice(col_off, col_off + 64)
                    nc.tensor.matmul(ps4[sl], hcol(f"{pset}_dwT"), x3q[:],
                                     start=True, stop=True,
                                     tile_position=(0, col_off))
                    so = hq.tile([128, BL], F32, name=f"{name}_so", tag="st_so",
                                 bufs=10)
                    nc.scalar.activation(so[sl], ps4[sl], AF.Relu,
                                         bias=bh_(f"{pset}_dwb", col_off, 64),
                                         scale=1.0)
                    return so

                vmat = {}
                for (i, j), pset in (((0, 0), "vc"), ((0, 2), "vc"), ((2, 0), "vc"),
                                     ((2, 2), "vc"), ((0, 1), "ve"), ((1, 0), "ve"),
                                     ((1, 2), "ve"), ((2, 1), "ve"), ((1, 1), "vm")):
                    vmat[(i, j)] = star(regq[(i, j)], pset, 0, f"v{i}{j}")

                def avg2(a_ap, b_ap, name):
                    s = hq.tile([64, BL], F32, name=f"{name}_s", tag="av_s")
                    nc.vector.tensor_tensor(out=s[:], in0=a_ap, in1=b_ap, op=ALU.add)
                    y = hq.tile([64, BL], F32, name=f"{name}_y", tag="av_y")
                    nc.vector.tensor_scalar(out=y[:], in0=s[:], scalar1=0.5,
                                            scalar2=127.0, op0=ALU.mult, op1=ALU.min)
                    nc.vector.tensor_scalar(out=y[:], in0=y[:], scalar1=-128.0,
                                            scalar2=None, op0=ALU.max)
                    r = hq.tile([64, BL], F32, name=f"{name}_r", tag="av_r")
                    nc.vector.tensor_scalar(out=r[:], in0=y[:], scalar1=MAGIC,
                                            scalar2=MAGIC, op0=ALU.add,
                                            op1=ALU.subtract)
                    g = hq.tile([64, BL], F32, name=f"{name}_g", tag="av_g")
                    nc.vector.tensor_tensor(out=g[:], in0=r[:], in1=y[:],
                                            op=ALU.is_gt)
                    q = hq.tile([64, BL], F32, name=f"{name}_q", tag="av_q")
                    nc.vector.tensor_tensor(out=q[:], in0=r[:], in1=g[:],
                                            op=ALU.subtract)
                    return q

                def avg4(i0, i1, i2, i3, name):
                    ab = avg2(vmat[i0][0:64], vmat[i1][0:64], f"{name}ab")
                    cd = avg2(vmat[i2][0:64], vmat[i3][0:64], f"{name}cd")
                    fq = avg2(ab[:], cd[:], f"{name}f")
                    q16 = hq.tile([64, BL], F16, name=f"{name}_q16", tag="av_q16")
                    nc.vector.tensor_copy(out=q16[:], in_=fq[:])
                    return q16

                a00 = avg4((0, 0), (0, 1), (1, 0), (1, 1), "a00")
                a01 = avg4((0, 1), (0, 2), (1, 1), (1, 2), "a01")
                a10 = avg4((1, 0), (1, 1), (2, 0), (2, 1), "a10")
                a11 = avg4((1, 1), (1, 2), (2, 1), (2, 2), "a11")

                q00 = star(a00, "vq", 64, "q00")
                q01 = star(a01, "vq", 0, "q01")
                q10 = star(a10, "vq", 64, "q10")
                q11 = star(a11, "vq", 0, "q11")

                vcat = hs.tile([128, 3, BL], F16)
                nc.vector.tensor_copy(out=vcat[0:64, 0, :], in_=pwin[:])
                for qv, r0, ci in ((q00, 64, 0), (q01, 0, 1),
                                   (q10, 64, 1), (q11, 0, 2)):
                    t1 = hq.tile([128, BL], F32, name="vqt", tag="vq_t")
                    sl = slice(r0, r0 + 64)
                    nc.vector.tensor_scalar(out=t1[sl], in0=qv[sl], scalar1=127.0,
                                            scalar2=MAGIC, op0=ALU.min, op1=ALU.add)
                    nc.vector.tensor_scalar(out=vcat[sl, ci, :], in0=t1[sl],
                                            scalar1=MAGIC, scalar2=None,
                                            op0=ALU.subtract)

                psv = hp.tile([64, BL], F32, name="psv", tag="psv")
                nc.tensor.matmul(psv[:], hcol("vl1T0"), vcat[:, 0, :],
                                 start=True, stop=False)
                nc.tensor.matmul(psv[:], hcol("vl1T1"), vcat[:, 1, :],
                                 start=False, stop=False)
                nc.tensor.matmul(psv[:], hcol("vl1T2"), vcat[0:64, 2, :],
                                 start=False, stop=True)
                v1a = hq.tile([64, BL], F32, name="v1a", tag="v1a")
                nc.scalar.activation(v1a[:], psv[:], AF.Relu,
                                     bias=bh_("vl1b", 0, 64), scale=1.0)
                v1q = hq.tile([64, BL], F16, name="v1q", tag="v1q")
                nc.vector.tensor_scalar(out=v1a[:], in0=v1a[:], scalar1=127.0,
                                        scalar2=MAGIC, op0=ALU.min, op1=ALU.add)
                nc.vector.tensor_scalar(out=v1q[:], in0=v1a[:], scalar1=MAGIC,
                                        scalar2=None, op0=ALU.subtract)

                psv2 = hp.tile([64, BL], F32, name="psv2", tag="psv")
                nc.tensor.matmul(psv2[:], hcol("vl2T"), v1q[:], start=True, stop=True)
                v2a = hq.tile([64, BL], F32, name="v2a", tag="v1a")
                nc.scalar.activation(v2a[:], psv2[:], AF.Relu,
                                     bias=bh_("vl2b", 0, 64), scale=1.0)
                v2q = hq.tile([64, BL], F16, name="v2q", tag="v1q")
                nc.vector.tensor_scalar(out=v2a[:], in0=v2a[:], scalar1=127.0,
                                        scalar2=MAGIC, op0=ALU.min, op1=ALU.add)
                nc.vector.tensor_scalar(out=v2q[:], in0=v2a[:], scalar1=MAGIC,
                                        scalar2=None, op0=ALU.subtract)

                psv3 = hp.tile([3, BL], F32, name="psv3", tag="psv")
                nc.tensor.matmul(psv3[:], hcol("vl3T"), v2q[:], start=True, stop=True)
                vout = hs.tile([3, BL], F32)
                nc.scalar.activation(vout[:], psv3[:], AF.Identity,
                                     bias=bh_("vl3b", 0, 3), scale=1.0 / 128.0)
                nc.sync.dma_start(out=value_d[:], in_=vout[:])

                # ---- policy
                psp1 = hp.tile([64, BL], F32, name="psp1", tag="psv")
                nc.tensor.matmul(psp1[:], hcol("pw1T"), pwin[:], start=True, stop=True)
                pwa = hq.tile([64, BL], F32, name="pwa", tag="v1a")
                nc.scalar.activation(pwa[:], psp1[:], AF.Relu,
                                     bias=bh_("pw1b", 0, 64), scale=1.0)
                pw1q = hq.tile([64, BL], F16, name="pw1q", tag="v1q")
                nc.vector.tensor_scalar(out=pwa[:], in0=pwa[:], scalar1=127.0,
                                        scalar2=MAGIC, op0=ALU.min, op1=ALU.add)
                nc.vector.tensor_scalar(out=pw1q[:], in0=pwa[:], scalar1=MAGIC,
                                        scalar2=None, op0=ALU.subtract)

                pwct = hs.tile([32, 16 * BL], F32)   # [c, b*16+m]
                pwcb = hs.tile([16, BL], F32)
                c0pw2, _ = hmap["pw2T"]
                for j in range(5):
                    rows = 128 if j < 4 else 16
                    pspj = hp.tile([128, BL], F32, name=f"pspj{j}", tag="st_p1")
                    nc.tensor.matmul(pspj[0:rows],
                                     whead[0:64, c0pw2 + 128 * j:c0pw2 + 128 * j + rows],
                                     pw1q[:], start=True, stop=True)
                    yj = hq.tile([128, BL], F32, name=f"yj{j}", tag="st_v1")
                    nc.scalar.activation(yj[0:rows], pspj[0:rows], AF.Identity,
                                         bias=bh_(f"pw2b{j}", 0, rows), scale=1.0)
                    if j < 4:
                        nc.vector.tensor_scalar(out=yj[0:rows], in0=yj[0:rows],
                                                scalar1=128.0, scalar2=32767.0,
                                                op0=ALU.mult, op1=ALU.min)
                        nc.vector.tensor_scalar(out=yj[0:rows], in0=yj[0:rows],
                                                scalar1=-32768.0, scalar2=None,
                                                op0=ALU.max)
                        for mr in range(4):
                            m = 4 * j + mr
                            nc.sync.dma_start(
                                out=pwct[:, m::16],
                                in_=yj[32 * mr:32 * mr + 32, :],
                            )
                    else:
                        nc.vector.tensor_scalar(out=pwcb[:], in0=yj[0:16],
                                                scalar1=16384.0, scalar2=None,
                                                op0=ALU.mult)
                if debug:
                    nc.sync.dma_start(out=dram["pwc_dbg"][:], in_=pwct[:])

                fdqv = fdq[:].rearrange("p (b f) -> p b f", f=HW)
                for b in range(BL):
                    pspol = hp.tile([16, HW], F32, name="pspol", tag="pspol", bufs=2)
                    nc.tensor.matmul(pspol[:], pwct[:, 16 * b:16 * (b + 1)],
                                     fdqv[:, b, :], start=True, stop=True)
                    polr = hq.tile([16, HW], F32, name="polr", tag="polr")
                    nc.scalar.activation(polr[:], pspol[:], AF.Relu,
                                         bias=pwcb[:, b:b + 1], scale=1.0)
                    pso = hp.tile([1, HW], F32, name="pso", tag="pso", bufs=1)
                    nc.tensor.matmul(pso[:], wheadf[0:16, 64:65], polr[:],
                                     start=True, stop=True)
                    pout = hq.tile([1, HW], F32, name="pout", tag="pout")
                    nc.scalar.activation(pout[:], pso[:], AF.Identity,
                                         bias=bh_("poutb", 0, 1), scale=1.0 / (2.0**21))
                    nc.sync.dma_start(out=policy_d[b:b + 1, :], in_=pout[:])

    split_waits(nc)
    return nc, dram


# ---------------------------------------------------------------- entry point

def _run(board_input, params, debug=False, trace=False):
    board_input = np.asarray(board_input, np.float32)
    B = board_input.shape[0]
    assert B == N_CORES * BL

    wd, tmap, bmap, hmap, bhmap = prep_weights(params)
    shapes = {k: list(v.shape) for k, v in wd.items()}

    key = ("prog", debug)
    if key not in _prog_cache:
        _prog_cache[key] = build_program(tmap, bmap, hmap, bhmap, shapes, debug=debug)
    nc, dram = _prog_cache[key]

    boards = board_input.transpose(1, 0, 2, 3).astype(np.float16)  # [2, B, 15, 15]
    boards = np.pad(boards, ((0, 0), (0, 0), (1, 1), (1, 1)))       # [2, B, 17, 17]
    in_maps = []
    for c in range(N_CORES):
        m = dict(wd)
        m["board"] = np.ascontiguousarray(
            boards[:, c * BL:(c + 1) * BL].reshape(2, BL * S))
        in_maps.append(m)

    res = run_bass_kernel_spmd(nc, in_maps, list(range(N_CORES)), trace=trace)

    value = np.zeros((B, 3), np.float32)
    policy = np.zeros((B, 1, 15, 15), np.float32)
    for c in range(N_CORES):
        r = res.results[c]
        value[c * BL:(c + 1) * BL] = r["value"].T
        policy[c * BL:(c + 1) * BL, 0] = r["policy"].reshape(BL, 15, 15)
    return (value, policy), res


def kernel(board_input, params):
    (value, policy), _ = _run(board_input, params)
    return value, policy


# revision 7
# speedup vs baseline: 1.0278x; 1.0278x over previous
"""Mix9Net forward pass on 8 Trainium2 NeuronCores (Bass/Tile).

Data parallel over batch (256 -> 32 per core). The conv trunk (4 directional
silu-resnets) runs in fp16 storage with fp32 PSUM accumulation; matmuls read
interior pixels through strided APs over zero-ring padded per-sample layouts.
The quantized heads run in exact integer arithmetic (values held at x128
integer scale); fake_quant's round() is the fp32 magic-number RNE trick and
floor() is RNE plus an is_gt fixup. Weight preprocessing (quantization,
transposes, scale folding) happens on host with jax-identical semantics.
"""
import sys

sys.path.insert(0, "/opt/trn_rl_repo")

import numpy as np

import concourse.bass as bass
import concourse.tile as tile
from concourse import mybir
from concourse.bass_utils import run_bass_kernel_spmd
from concourse.vector_clock import ScopedClock, VectorClock

F32 = mybir.dt.float32
F16 = mybir.dt.float16
AF = mybir.ActivationFunctionType
ALU = mybir.AluOpType
AX = mybir.AxisListType

MAGIC = float(np.float32(1.5 * 2.0**23))

N_CORES = 8
BL = 32                   # samples per core
S = 289                   # 17*17 padded sample
SLK = 18                  # head slack
PADW = SLK + BL * S + 20  # padded-flat width per partition
HW = 225

DIR_POS = (((1, 0), (1, 1), (1, 2)),
           ((0, 1), (1, 1), (2, 1)),
           ((0, 0), (1, 1), (2, 2)),
           ((2, 0), (1, 1), (0, 2)))

# ---------------------------------------------------------------- walrus glue


class TC(tile.TileContext):
    """This container's walrus accepts at most ONE sync wait per instruction;
    emit the kernel-tail drain as a chain of single-wait drains."""

    def _drain_and_barrier(self, tick_clock, wait_clock):
        gc = tick_clock.global_clock
        n = len(gc)
        for i in range(n):
            if gc[i] <= 0:
                continue
            vec = [0] * n
            vec[i] = gc[i]
            d = self.nc.sync.drain()
            wait_clock.add_sem_waits(d.ins, ScopedClock({None: VectorClock(vec)}))
        self.nc.all_engine_barrier()
        popped = self.nc._tile_sem_poison_stack.pop()
        assert popped is self._sem_poison
        self.nc.clear_and_free_semaphores(list(self.sems.allocated().values()))
        self.nc.all_engine_barrier()


def split_waits(nc):
    """Move excess sync waits onto same-engine NOPs placed just before the
    overloaded instruction (program order preserved -> semantics unchanged)."""
    for f in nc.m.functions:
        for bb in f.blocks:
            out = []
            for inst in bb.instructions:
                si = inst.sync_info
                if si is not None:
                    budget = max(0, min(1, 2 - len(si.on_update)))
                    if len(si.on_wait) > budget:
                        waits = list(si.on_wait)
                        nkeep = len(waits) - budget
                        extra, keep = waits[:nkeep], waits[nkeep:]
                        for i, w in enumerate(extra):
                            nop = mybir.InstNoOp(name=f"{inst.name}-wsp{i}",
                                                 engine=inst.engine)
                            nop.sync_info = mybir.SyncInfo(on_wait=[w], on_update=[])
                            out.append(nop)
                        inst.sync_info = mybir.SyncInfo(on_wait=list(keep),
                                                        on_update=list(si.on_update))
                out.append(inst)
            bb.instructions[:] = out


# ---------------------------------------------------------------- host prep

def _fq(x, scale, bits, floor=False):
    x = np.asarray(x, np.float32)
    qmin = np.float32(-(2.0 ** (bits - 1)))
    qmax = np.float32(2.0 ** (bits - 1) - 1)
    y = np.clip(x * np.float32(scale), qmin, qmax)
    q = np.floor(y) if floor else np.round(y)
    return (q.astype(np.float32) / np.float32(scale)).astype(np.float32)


def prep_weights(params):
    out = {}
    tmap = {}
    tcols = []

    def addt(name, a):
        tmap[name] = (sum(x.shape[1] for x in tcols), a.shape[1])
        tcols.append(np.ascontiguousarray(np.asarray(a, np.float32)))

    for mi, mkey in enumerate(("map1", "map2")):
        p = params[mkey]
        for bi, blk in enumerate(p["blocks"]):
            for k in range(3):
                addt(f"dcw{mi}_{bi}_{k}", np.asarray(blk["dcw"], np.float32)[k].T)
            addt(f"cw{mi}_{bi}", np.asarray(blk["cw"], np.float32).T)
        addt(f"c01_{mi}", np.asarray(p["c0_1w"], np.float32).T)
        addt(f"c02_{mi}", np.asarray(p["c0_2w"], np.float32).T)
        addt(f"fw_{mi}", np.asarray(p["fw"], np.float32).T)
    ncol = sum(a.shape[1] for a in tcols)
    wtrunk = np.zeros((128, ncol), np.float16)
    c = 0
    for a in tcols:
        wtrunk[: a.shape[0], c:c + a.shape[1]] = a.astype(np.float16)
        c += a.shape[1]
    out["wtrunk"] = wtrunk

    w18 = np.zeros((18, 4 * 128), np.float32)
    for d in range(4):
        mkey = "map1" if d < 2 else "map2"
        dw = np.asarray(params[mkey]["dw"], np.float32)  # [3, 128, 2]
        for k, (i, j) in enumerate(DIR_POS[d]):
            s = ((i - 1) + 1) * 3 + ((j - 1) + 1)
            for cc in range(2):
                w18[s * 2 + cc, d * 128:(d + 1) * 128] += dw[k, :, cc]
    out["w18t"] = w18.astype(np.float16)

    bmap = {}
    bt = np.zeros((128, 24), np.float32)
    bc = [0]

    def addb(name, v):
        bmap[name] = bc[0]
        v = np.asarray(v, np.float32)
        bt[: len(v), bc[0]] = v
        bc[0] += 1

    for mi, mkey in enumerate(("map1", "map2")):
        p = params[mkey]
        addb(f"db{mi}", p["db"])
        for bi, blk in enumerate(p["blocks"]):
            addb(f"dcb{mi}_{bi}", blk["dcb"])
            addb(f"cb{mi}_{bi}", blk["cb"])
        addb(f"c01b{mi}", p["c0_1b"])
        addb(f"c02b{mi}", p["c0_2b"])
        addb(f"fb{mi}", p["fb"])
    out["btrunk"] = bt

    wq = _fq(params["dwconv_w"], 65536, 16)
    wdw = np.zeros((64, 288), np.float16)
    for i in range(3):
        for j in range(3):
            k = 3 * i + j
            wdw[32:64, 32 * k:32 * k + 32] = (np.diag(wq[:, 0, i, j]) / 32.0
                                              ).astype(np.float16)
    out["wdw"] = wdw

    hmap = {}
    hcols = []

    def addh(name, a):
        hmap[name] = (sum(x.shape[1] for x in hcols), a.shape[1])
        hcols.append(np.asarray(a, np.float32))

    addh("pw1T", _fq(params["pw1_w"], 128, 8).T)
    addh("pw2T", _fq(params["pw2_w"], 128, 8).T)
    for sk in ("vc", "ve", "vm", "vq"):
        sp = params[sk]
        addh(f"{sk}_u1T", _fq(sp["u1w"], 128, 8).T)
        addh(f"{sk}_u2T", _fq(sp["u2w"], 128, 8).T)
        addh(f"{sk}_dwT", _fq(sp["dw"], 128, 8).T)
    vl1 = _fq(params["vl1_w"], 128, 8).T
    addh("vl1T0", vl1[0:128])
    addh("vl1T1", vl1[128:256])
    addh("vl1T2", vl1[256:320])
    addh("vl2T", _fq(params["vl2_w"], 128, 8).T)
    addh("vl3T", _fq(params["vl3_w"], 128, 8).T)
    ncol = sum(a.shape[1] for a in hcols)
    whead = np.zeros((128, ncol), np.float16)
    c = 0
    for a in hcols:
        whead[: a.shape[0], c:c + a.shape[1]] = a.astype(np.float16)
        c += a.shape[1]
    out["whead"] = whead

    whf = np.zeros((128, 65), np.float32)
    for j in range(64):
        whf[2 * j, j] = 1.0
        whf[2 * j + 1, j] = 1.0
    whf[0:16, 64] = np.asarray(params["pout_w"], np.float32)[0]
    out["wheadf"] = whf

    bhmap = {}
    bh = np.zeros((128, 24), np.float32)
    hc = [0]

    def addbh(name, v, dup64=False):
        bhmap[name] = hc[0]
        v = np.asarray(v, np.float32).reshape(-1)
        bh[: len(v), hc[0]] = v
        if dup64:
            bh[64:64 + len(v), hc[0]] = v
        hc[0] += 1

    for sk in ("vc", "ve", "vm", "vq"):
        sp = params[sk]
        addbh(f"{sk}_u1b", _fq(sp["u1b"], 128 * 128, 32) * 128)
        addbh(f"{sk}_u2b", _fq(sp["u2b"], 128 * 128, 32) * 128)
        addbh(f"{sk}_dwb", _fq(sp["db"], 128 * 128, 32) * 128, dup64=True)
    addbh("pw1b", _fq(params["pw1_b"], 128 * 128, 32) * 128)
    addbh("vl1b", _fq(params["vl1_b"], 128 * 128, 32) * 128)
    addbh("vl2b", _fq(params["vl2_b"], 128 * 128, 32) * 128)
    addbh("vl3b", _fq(params["vl3_b"], 128 * 128, 32))
    addbh("poutb", params["pout_b"])
    addbh("dwb", _fq(params["dwconv_b"], 128, 16) * 128)
    pw2b = _fq(params["pw2_b"], 128 * 128, 32) * 128
    for j in range(5):
        addbh(f"pw2b{j}", pw2b[128 * j:min(528, 128 * (j + 1))])
    out["bhead"] = bh

    return out, tmap, bmap, hmap, bhmap


# ---------------------------------------------------------------- device build

_prog_cache = {}


def build_program(tmap, bmap, hmap, bhmap, shapes, debug=False):
    nc = bass.Bass()
    dram = {}

    def din(name, shape, dt):
        dram[name] = nc.dram_tensor(name, list(shape), dt, kind="ExternalInput")
        return dram[name]

    def dout(name, shape, dt=F32):
        dram[name] = nc.dram_tensor(name, list(shape), dt, kind="ExternalOutput")
        return dram[name]

    board_d = din("board", [2, BL * S], F16)  # host-padded 17x17
    w18_d = din("w18t", shapes["w18t"], F16)
    wtrunk_d = din("wtrunk", shapes["wtrunk"], F16)
    btrunk_d = din("btrunk", shapes["btrunk"], F32)
    wdw_d = din("wdw", shapes["wdw"], F16)
    whead_d = din("whead", shapes["whead"], F16)
    wheadf_d = din("wheadf", shapes["wheadf"], F32)
    bhead_d = din("bhead", shapes["bhead"], F32)
    value_d = dout("value", [3, BL])
    policy_d = dout("policy", [BL, HW])
    if debug:
        dout("facc_dbg", [64, 7200])
        dout("fdq_dbg", [32, 7200])
        dout("fsum_dbg", [64, BL])
        dout("pwc_dbg", [32, 16 * BL])
        dout("xa_dbg", [128, PADW])

    with TC(nc) as tc:
        with tc.tile_pool(name="wpool", bufs=1) as wpool:
            # ---- persistent tiles
            xA = wpool.tile([128, PADW], F16)
            xB = wpool.tile([128, PADW], F16)
            X18 = wpool.tile([18, PADW], F16)
            PAD1 = wpool.tile([128, PADW], F16)  # r0-1 board, r32-63 fdin, r64-127 frel
            facc = wpool.tile([64, 7200], F32)
            fdq = wpool.tile([32, 7200], F32)
            wtrunk = wpool.tile([128, shapes["wtrunk"][1]], F16)
            w18t = wpool.tile([18, shapes["w18t"][1]], F16)
            btrunk = wpool.tile([128, shapes["btrunk"][1]], F32)
            wdw = wpool.tile([64, shapes["wdw"][1]], F16)
            whead = wpool.tile([128, shapes["whead"][1]], F16)
            wheadf = wpool.tile([128, shapes["wheadf"][1]], F32)
            bhead = wpool.tile([128, shapes["bhead"][1]], F32)

            nc.vector.memset(xA[:], 0.0)
            nc.gpsimd.memset(xB[:], 0.0)
            nc.vector.memset(PAD1[:], 0.0)

            nc.gpsimd.dma_start(out=wtrunk[:], in_=wtrunk_d[:])
            nc.gpsimd.dma_start(out=w18t[:], in_=w18_d[:])
            nc.gpsimd.dma_start(out=btrunk[:], in_=btrunk_d[:])
            nc.gpsimd.dma_start(out=wdw[:], in_=wdw_d[:])
            nc.gpsimd.dma_start(out=whead[:], in_=whead_d[:])
            nc.gpsimd.dma_start(out=wheadf[:], in_=wheadf_d[:])
            nc.gpsimd.dma_start(out=bhead[:], in_=bhead_d[:])

            def padview(t):  # [128, BL, 17, 17]
                return t[:, SLK:SLK + BL * S].rearrange(
                    "p (b h w) -> p b h w", h=17, w=17)

            def interior(t, b0, nb):  # [*, nb, 15, 15]
                return padview(t)[:, b0:b0 + nb, 1:16, 1:16]

            def interior5(t, g):  # [*, 4, 2, 15, 15] for group g (8 samples)
                v = t[:, SLK + 8 * g * S: SLK + (8 * g + 8) * S].rearrange(
                    "p (a b h w) -> p a b h w", a=4, b=2, h=17, w=17)
                return v[:, :, :, 1:16, 1:16]

            pv1 = padview(PAD1)
            nc.gpsimd.dma_start(out=PAD1[0:2, SLK:SLK + BL * S], in_=board_d[:])

            for s in range(9):
                di, dj = s // 3 - 1, s % 3 - 1
                sh = di * 17 + dj
                nc.sync.dma_start(
                    out=X18[2 * s:2 * s + 2, SLK:SLK + BL * S],
                    in_=PAD1[0:2, SLK + sh:SLK + sh + BL * S],
                )
            x18v = padview(X18)

            def tb(name):
                return btrunk[:, bmap[name]:bmap[name] + 1]

            def wcol(name):
                c0, w = tmap[name]
                return wtrunk[:, c0:c0 + w]

            with tc.tile_pool(name="ppool", bufs=2, space="PSUM") as ppool:
                # ================= trunk =================
                for d in range(4):
                    mi = d // 2
                    shifts = [((i - 1), (j - 1)) for (i, j) in DIR_POS[d]]
                    cur, nxt = (xA, xB)

                    for g in range(4):
                        ps = ppool.tile([128, 4, 512], F32, name="ps1", tag="ps")
                        for pr in range(4):
                            b0 = 8 * g + 2 * pr
                            nc.tensor.matmul(
                                ps[:, pr, 0:450],
                                w18t[:, d * 128:(d + 1) * 128],
                                x18v[:, b0:b0 + 2, 1:16, 1:16],
                                start=True, stop=True,
                            )
                        nc.scalar.activation(
                            interior5(cur, g),
                            ps[:, :, 0:450].rearrange(
                                "p a (b h w) -> p a b h w", b=2, h=15, w=15),
                            AF.Silu, bias=tb(f"db{mi}"), scale=1.0,
                        )

                    for bi in range(4):
                        for g in range(4):
                            ps = ppool.tile([128, 4, 512], F32, name="psd", tag="ps")
                            for pr in range(4):
                                b0 = 8 * g + 2 * pr
                                for k in range(3):
                                    di, dj = shifts[k]
                                    rhs = padview(cur)[:, b0:b0 + 2,
                                                       1 + di:16 + di, 1 + dj:16 + dj]
                                    nc.tensor.matmul(
                                        ps[:, pr, 0:450],
                                        wcol(f"dcw{mi}_{bi}_{k}"),
                                        rhs,
                                        start=(k == 0), stop=(k == 2),
                                    )
                            tg = wpool.tile([128, 1800], F16, name="tg",
                                            tag="tg", bufs=2)
                            nc.scalar.activation(
                                tg[:].rearrange("p (a b) -> p a b", a=4),
                                ps[:, :, 0:450],
                                AF.Silu, bias=tb(f"dcb{mi}_{bi}"), scale=1.0,
                            )
                            ps2 = ppool.tile([128, 4, 512], F32, name="psc", tag="ps")
                            for pr in range(4):
                                nc.tensor.matmul(
                                    ps2[:, pr, 0:450],
                                    wcol(f"cw{mi}_{bi}"),
                                    tg[:, 450 * pr:450 * (pr + 1)],
                                    start=True, stop=True,
                                )
                            tsil = wpool.tile([128, 1800], F16, name="tsil",
                                              tag="tsil", bufs=3)
                            nc.scalar.activation(
                                tsil[:].rearrange("p (a b) -> p a b", a=4),
                                ps2[:, :, 0:450],
                                AF.Silu, bias=tb(f"cb{mi}_{bi}"), scale=1.0,
                            )
                            nc.vector.tensor_tensor(
                                out=interior(nxt, 8 * g, 8),
                                in0=tsil[:].rearrange("p (b h w) -> p b h w",
                                                      h=15, w=15),
                                in1=interior(cur, 8 * g, 8),
                                op=ALU.add,
                            )
                        cur, nxt = nxt, cur

                    for g in range(4):
                        ps = ppool.tile([128, 4, 512], F32, name="psu", tag="ps")
                        for pr in range(4):
                            b0 = 8 * g + 2 * pr
                            nc.tensor.matmul(
                                ps[:, pr, 0:450], wcol(f"c01_{mi}"),
                                interior(cur, b0, 2), start=True, stop=True,
                            )
                        u1 = wpool.tile([128, 1800], F16, name="u1", tag="tg", bufs=2)
                        nc.scalar.activation(
                            u1[:].rearrange("p (a b) -> p a b", a=4),
                            ps[:, :, 0:450],
                            AF.Silu, bias=tb(f"c01b{mi}"), scale=1.0,
                        )
                        ps2 = ppool.tile([128, 4, 512], F32, name="psu2", tag="ps")
                        for pr in range(4):
                            nc.tensor.matmul(
                                ps2[:, pr, 0:450], wcol(f"c02_{mi}"),
                                u1[:, 450 * pr:450 * (pr + 1)], start=True, stop=True,
                            )
                        tsil2 = wpool.tile([128, 1800], F16, name="tsil2",
                                           tag="tsil", bufs=3)
                        nc.scalar.activation(
                            tsil2[:].rearrange("p (a b) -> p a b", a=4),
                            ps2[:, :, 0:450],
                            AF.Silu, bias=tb(f"c02b{mi}"), scale=1.0,
                        )
                        x5 = wpool.tile([128, 1800], F16, name="x5",
                                        tag="tsil", bufs=3)
                        nc.vector.tensor_tensor(
                            out=x5[:].rearrange("p (b h w) -> p b h w", h=15, w=15),
                            in0=tsil2[:].rearrange("p (b h w) -> p b h w", h=15, w=15),
                            in1=interior(cur, 8 * g, 8),
                            op=ALU.add,
                        )
                        psF = ppool.tile([64, 4, 512], F32, name="psf", tag="ps")
                        for pr in range(4):
                            nc.tensor.matmul(
                                psF[:, pr, 0:450], wcol(f"fw_{mi}"),
                                x5[:, 450 * pr:450 * (pr + 1)], start=True, stop=True,
                            )
                        z = wpool.tile([64, 1800], F32, name="z", tag="z", bufs=2)
                        zv = z[:].rearrange("p (a b) -> p a b", a=4)
                        nc.vector.tensor_scalar(
                            out=zv, in0=psF[:, :, 0:450],
                            scalar1=btrunk[0:64, bmap[f"fb{mi}"]:bmap[f"fb{mi}"] + 1],
                            scalar2=32.0, op0=ALU.add, op1=ALU.mult,
                        )
                        nc.gpsimd.tensor_scalar(
                            out=z[:], in0=z[:], scalar1=512.0, scalar2=-512.0,
                            op0=ALU.min, op1=ALU.max,
                        )
                        fslice = facc[:, 1800 * g:1800 * (g + 1)]
                        if d == 0:
                            nc.gpsimd.tensor_scalar(
                                out=fslice, in0=z[:], scalar1=MAGIC, scalar2=MAGIC,
                                op0=ALU.add, op1=ALU.subtract,
                            )
                        else:
                            nc.gpsimd.tensor_scalar(
                                out=z[:], in0=z[:], scalar1=MAGIC, scalar2=MAGIC,
                                op0=ALU.add, op1=ALU.subtract,
                            )
                            nc.vector.tensor_tensor(out=fslice, in0=z[:],
                                                    in1=fslice, op=ALU.add)

                # ============== feature stage ==============
                # frel (x128 ints, relu) -> PAD1 rows 64:128; fdin -> PAD1 rows 32:64
                nc.scalar.activation(PAD1[64:128, 0:7200], facc[:], AF.Relu,
                                     bias=0.0, scale=1.0)
                nc.scalar.activation(
                    pv1[32:64, :, 1:16, 1:16],
                    facc[0:32, :].rearrange("p (b h w) -> p b h w", h=15, w=15),
                    AF.Relu, bias=0.0, scale=1.0,
                )

                for g in range(4):
                    psdw = ppool.tile([32, 4, 512], F32, name="psdw", tag="ps")
                    for pr in range(4):
                        b0 = 8 * g + 2 * pr
                        for k in range(9):
                            di, dj = k // 3 - 1, k % 3 - 1
                            rhs = pv1[32:64, b0:b0 + 2,
                                      1 + di:16 + di, 1 + dj:16 + dj]
                            nc.tensor.matmul(
                                psdw[:, pr, 0:450],
                                wdw[32:64, 32 * k:32 * k + 32],
                                rhs,
                                start=(k == 0), stop=(k == 8),
                                tile_position=(32, 0),
                            )
                    fdt = wpool.tile([32, 1800], F32, name="fdt", tag="z", bufs=2)
                    nc.scalar.activation(
                        fdt[:].rearrange("p (a b) -> p a b", a=4),
                        psdw[:, :, 0:450], AF.Relu,
                        bias=bhead[0:32, bhmap["dwb"]:bhmap["dwb"] + 1], scale=128.0,
                    )
                    nc.gpsimd.tensor_scalar(
                        out=fdt[:], in0=fdt[:], scalar1=32767.0, scalar2=MAGIC,
                        op0=ALU.min, op1=ALU.add,
                    )
                    nc.gpsimd.tensor_scalar(
                        out=fdq[:, 1800 * g:1800 * (g + 1)], in0=fdt[:],
                        scalar1=MAGIC, scalar2=None, op0=ALU.subtract,
                    )

            if debug:
                nc.sync.dma_start(out=dram["facc_dbg"][:], in_=facc[:])
                nc.sync.dma_start(out=dram["fdq_dbg"][:], in_=fdq[:])
                nc.gpsimd.dma_start(out=dram["xa_dbg"][:], in_=xA[:])

            # ================= heads =================
            with (
                tc.tile_pool(name="hq", bufs=4) as hq,
                tc.tile_pool(name="hs", bufs=1) as hs,
                tc.tile_pool(name="hp", bufs=1, space="PSUM") as hp,
            ):
                frel = PAD1[64:128, 0:7200]

                fsumS = hs.tile([128, BL], F32)
                nc.vector.tensor_reduce(
                    out=fsumS[0:32, :],
                    in_=fdq[:].rearrange("p (b f) -> p b f", f=HW),
                    axis=AX.X, op=ALU.add,
                )
                nc.vector.tensor_reduce(
                    out=fsumS[96:128, :],
                    in_=frel[32:64, :].rearrange("p (b f) -> p b f", f=HW),
                    axis=AX.X, op=ALU.add,
                )
                nc.sync.dma_start(out=fsumS[32:64, :], in_=fsumS[96:128, :])

                def floor_int(src_ap, pre_mult, name):
                    """floor(src*pre_mult) over [64, BL] -> f32 ints."""
                    y = hq.tile([64, BL], F32, name=f"{name}_y", tag="fl_y")
                    nc.vector.tensor_scalar(out=y[:], in0=src_ap, scalar1=pre_mult,
                                            scalar2=None, op0=ALU.mult)
                    r = hq.tile([64, BL], F32, name=f"{name}_r", tag="fl_r")
                    nc.vector.tensor_scalar(out=r[:], in0=y[:], scalar1=MAGIC,
                                            scalar2=MAGIC, op0=ALU.add,
                                            op1=ALU.subtract)
                    gt = hq.tile([64, BL], F32, name=f"{name}_g", tag="fl_g")
                    nc.vector.tensor_tensor(out=gt[:], in0=r[:], in1=y[:],
                                            op=ALU.is_gt)
                    nc.vector.tensor_tensor(out=r[:], in0=r[:], in1=gt[:],
                                            op=ALU.subtract)
                    return r

                fsum_i = floor_int(fsumS[0:64, :], 1.0 / 256.0, "fsum")
                if debug:
                    nc.sync.dma_start(out=dram["fsum_dbg"][:], in_=fsum_i[:])

                pwin = hs.tile([64, BL], F16)
                nc.vector.tensor_scalar(out=pwin[:], in0=fsum_i[:], scalar1=127.0,
                                        scalar2=-128.0, op0=ALU.min, op1=ALU.max)

                hb = (0, 5, 10, 15)
                regq = {}
                for i in range(3):
                    for j in range(3):
                        rS = hq.tile([128, BL], F32, name=f"r{i}{j}", tag="regS")
                        nc.vector.tensor_reduce(
                            out=rS[0:32, :],
                            in_=fdq[:].rearrange("p (b h w) -> p b h w",
                                                 h=15, w=15)[
                                :, :, hb[i]:hb[i + 1], hb[j]:hb[j + 1]],
                            axis=AX.XY, op=ALU.add,
                        )
                        nc.vector.tensor_reduce(
                            out=rS[96:128, :],
                            in_=frel[32:64, :].rearrange("p (b h w) -> p b h w",
                                                         h=15, w=15)[
                                :, :, hb[i]:hb[i + 1], hb[j]:hb[j + 1]],
                            axis=AX.XY, op=ALU.add,
                        )
                        nc.sync.dma_start(out=rS[32:64, :], in_=rS[96:128, :])
                        ri = floor_int(rS[0:64, :], 1.0 / 32.0, f"ri{i}{j}")
                        q = hs.tile([64, BL], F16, name=f"regq{i}{j}",
                                    tag="regq", bufs=10)
                        nc.vector.tensor_scalar(out=q[:], in0=ri[:], scalar1=127.0,
                                                scalar2=-128.0, op0=ALU.min,
                                                op1=ALU.max)
                        regq[(i, j)] = q

                HROWS = {"pw1T": 64, "pw2T": 64, "vl1T0": 128, "vl1T1": 128,
                         "vl1T2": 64, "vl2T": 64, "vl3T": 64}

                def hcol(name):
                    c0, w = hmap[name]
                    rows = HROWS.get(name, 64)
                    return whead[0:rows, c0:c0 + w]

                def bh_(name, r0, rows):
                    return bhead[r0:r0 + rows, bhmap[name]:bhmap[name] + 1]

                def quant8_floor(v_ap, rows, relu, name):
                    c = hq.tile([128, BL], F32, name=f"{name}_c", tag="q8_c")
                    if relu:
                        nc.vector.tensor_scalar(out=c[0:rows], in0=v_ap,
                                                scalar1=127.0, scalar2=None,
                                                op0=ALU.min)
                    else:
                        nc.vector.tensor_scalar(out=c[0:rows], in0=v_ap,
                                                scalar1=127.0, scalar2=-128.0,
                                                op0=ALU.min, op1=ALU.max)
                    r = hq.tile([128, BL], F32, name=f"{name}_r", tag="q8_r")
                    nc.vector.tensor_scalar(out=r[0:rows], in0=c[0:rows],
                                            scalar1=MAGIC, scalar2=MAGIC,
                                            op0=ALU.add, op1=ALU.subtract)
                    g = hq.tile([128, BL], F32, name=f"{name}_g", tag="q8_g")
                    nc.vector.tensor_tensor(out=g[0:rows], in0=r[0:rows],
                                            in1=c[0:rows], op=ALU.is_gt)
                    q = hq.tile([128, BL], F16, name=f"{name}_q", tag="q8_q")
                    nc.vector.tensor_tensor(out=q[0:rows], in0=r[0:rows],
                                            in1=g[0:rows], op=ALU.subtract)
                    return q

                def star(xq, pset, col_off, name):
                    ps1 = hp.tile([128, BL], F32, name=f"{name}_p1", tag="st_p1")
                    nc.tensor.matmul(ps1[:], hcol(f"{pset}_u1T"), xq[0:64],
                                     start=True, stop=True)
                    v1 = hq.tile([128, BL], F32, name=f"{name}_v1", tag="st_v1")
                    nc.scalar.activation(v1[:], ps1[:], AF.Relu,
                                         bias=bh_(f"{pset}_u1b", 0, 128), scale=1.0)
                    x1q = quant8_floor(v1[:], 128, True, f"{name}_x1")
                    ps2 = hp.tile([128, BL], F32, name=f"{name}_p2", tag="st_p2")
                    nc.tensor.matmul(ps2[:], hcol(f"{pset}_u2T"), xq[0:64],
                                     start=True, stop=True)
                    v2 = hq.tile([128, BL], F32, name=f"{name}_v2", tag="st_v2")
                    nc.scalar.activation(v2[:], ps2[:], AF.Identity,
                                         bias=bh_(f"{pset}_u2b", 0, 128), scale=1.0)
                    x2q = quant8_floor(v2[:], 128, False, f"{name}_x2")
                    p = hq.tile([128, BL], F32, name=f"{name}_pp", tag="st_pp")
                    nc.vector.tensor_tensor(out=p[:], in0=x1q[:], in1=x2q[:],
                                            op=ALU.mult)
                    ps3 = hp.tile([64, BL], F32, name=f"{name}_p3", tag="st_p3")
                    nc.tensor.matmul(ps3[:], wheadf[:, 0:64], p[:],
                                     start=True, stop=True)
                    y3 = hq.tile([64, BL], F32, name=f"{name}_y3", tag="st_y3")
                    nc.vector.tensor_scalar(out=y3[:], in0=ps3[:],
                                            scalar1=1.0 / 128.0, scalar2=127.0,
                                            op0=ALU.mult, op1=ALU.min)
                    nc.vector.tensor_scalar(out=y3[:], in0=y3[:], scalar1=-128.0,
                                            scalar2=None, op0=ALU.max)
                    r3 = hq.tile([64, BL], F32, name=f"{name}_r3", tag="st_r3")
                    nc.vector.tensor_scalar(out=r3[:], in0=y3[:], scalar1=MAGIC,
                                            scalar2=MAGIC, op0=ALU.add,
                                            op1=ALU.subtract)
                    g3 = hq.tile([64, BL], F32, name=f"{name}_g3", tag="st_g3")
                    nc.vector.tensor_tensor(out=g3[:], in0=r3[:], in1=y3[:],
                                            op=ALU.is_gt)
                    x3q = hq.tile([64, BL], F16, name=f"{name}_x3", tag="st_x3")
                    nc.vector.tensor_tensor(out=x3q[:], in0=r3[:], in1=g3[:],
                                            op=ALU.subtract)
                    ps4 = hp.tile([128, BL], F32, name=f"{name}_p4", tag="st_p4")
                    sl = slice(col_off, col_off + 64)
                    nc.tensor.matmul(ps4[sl], hcol(f"{pset}_dwT"), x3q[:],
                                     start=True, stop=True,
                                     tile_position=(0, col_off))
                    so = hq.tile([128, BL], F32, name=f"{name}_so", tag="st_so",
                                 bufs=10)
                    nc.scalar.activation(so[sl], ps4[sl], AF.Relu,
                                         bias=bh_(f"{pset}_dwb", col_off, 64),
                                         scale=1.0)
                    return so

                vmat = {}
                for (i, j), pset in (((0, 0), "vc"), ((0, 2), "vc"), ((2, 0), "vc"),
                                     ((2, 2), "vc"), ((0, 1), "ve"), ((1, 0), "ve"),
                                     ((1, 2), "ve"), ((2, 1), "ve"), ((1, 1), "vm")):
                    vmat[(i, j)] = star(regq[(i, j)], pset, 0, f"v{i}{j}")

                def avg2(a_ap, b_ap, name):
                    s = hq.tile([64, BL], F32, name=f"{name}_s", tag="av_s")
                    nc.vector.tensor_tensor(out=s[:], in0=a_ap, in1=b_ap, op=ALU.add)
                    y = hq.tile([64, BL], F32, name=f"{name}_y", tag="av_y")
                    nc.vector.tensor_scalar(out=y[:], in0=s[:], scalar1=0.5,
                                            scalar2=127.0, op0=ALU.mult, op1=ALU.min)
                    nc.vector.tensor_scalar(out=y[:], in0=y[:], scalar1=-128.0,
                                            scalar2=None, op0=ALU.max)
                    r = hq.tile([64, BL], F32, name=f"{name}_r", tag="av_r")
                    nc.vector.tensor_scalar(out=r[:], in0=y[:], scalar1=MAGIC,
                                            scalar2=MAGIC, op0=ALU.add,
                                            op1=ALU.subtract)
                    g = hq.tile([64, BL], F32, name=f"{name}_g", tag="av_g")
                    nc.vector.tensor_tensor(out=g[:], in0=r[:], in1=y[:],
                                            op=ALU.is_gt)
                    q = hq.tile([64, BL], F32, name=f"{name}_q", tag="av_q")
                    nc.vector.tensor_tensor(out=q[:], in0=r[:], in1=g[:],
                                            op=ALU.subtract)
                    return q

                def avg4(i0, i1, i2, i3, name):
                    ab = avg2(vmat[i0][0:64], vmat[i1][0:64], f"{name}ab")
                    cd = avg2(vmat[i2][0:64], vmat[i3][0:64], f"{name}cd")
                    fq = avg2(ab[:], cd[:], f"{name}f")
                    q16 = hq.tile([64, BL], F16, name=f"{name}_q16", tag="av_q16")
                    nc.vector.tensor_copy(out=q16[:], in_=fq[:])
                    return q16

                a00 = avg4((0, 0), (0, 1), (1, 0), (1, 1), "a00")
                a01 = avg4((0, 1), (0, 2), (1, 1), (1, 2), "a01")
                a10 = avg4((1, 0), (1, 1), (2, 0), (2, 1), "a10")
                a11 = avg4((1, 1), (1, 2), (2, 1), (2, 2), "a11")

                q00 = star(a00, "vq", 64, "q00")
                q01 = star(a01, "vq", 0, "q01")
                q10 = star(a10, "vq", 64, "q10")
                q11 = star(a11, "vq", 0, "q11")

                vcat = hs.tile([128, 3, BL], F16)
                nc.vector.tensor_copy(out=vcat[0:64, 0, :], in_=pwin[:])
                for qv, r0, ci in ((q00, 64, 0), (q01, 0, 1),
                                   (q10, 64, 1), (q11, 0, 2)):
                    t1 = hq.tile([128, BL], F32, name="vqt", tag="vq_t")
                    sl = slice(r0, r0 + 64)
                    nc.vector.tensor_scalar(out=t1[sl], in0=qv[sl], scalar1=127.0,
                                            scalar2=MAGIC, op0=ALU.min, op1=ALU.add)
                    nc.vector.tensor_scalar(out=vcat[sl, ci, :], in0=t1[sl],
                                            scalar1=MAGIC, scalar2=None,
                                            op0=ALU.subtract)

                psv = hp.tile([64, BL], F32, name="psv", tag="psv")
                nc.tensor.matmul(psv[:], hcol("vl1T0"), vcat[:, 0, :],
                                 start=True, stop=False)
                nc.tensor.matmul(psv[:], hcol("vl1T1"), vcat[:, 1, :],
                                 start=False, stop=False)
                nc.tensor.matmul(psv[:], hcol("vl1T2"), vcat[0:64, 2, :],
                                 start=False, stop=True)
                v1a = hq.tile([64, BL], F32, name="v1a", tag="v1a")
                nc.scalar.activation(v1a[:], psv[:], AF.Relu,
                                     bias=bh_("vl1b", 0, 64), scale=1.0)
                v1q = hq.tile([64, BL], F16, name="v1q", tag="v1q")
                nc.vector.tensor_scalar(out=v1a[:], in0=v1a[:], scalar1=127.0,
                                        scalar2=MAGIC, op0=ALU.min, op1=ALU.add)
                nc.vector.tensor_scalar(out=v1q[:], in0=v1a[:], scalar1=MAGIC,
                                        scalar2=None, op0=ALU.subtract)

                psv2 = hp.tile([64, BL], F32, name="psv2", tag="psv")
                nc.tensor.matmul(psv2[:], hcol("vl2T"), v1q[:], start=True, stop=True)
                v2a = hq.tile([64, BL], F32, name="v2a", tag="v1a")
                nc.scalar.activation(v2a[:], psv2[:], AF.Relu,
                                     bias=bh_("vl2b", 0, 64), scale=1.0)
                v2q = hq.tile([64, BL], F16, name="v2q", tag="v1q")
                nc.vector.tensor_scalar(out=v2a[:], in0=v2a[:], scalar1=127.0,
                                        scalar2=MAGIC, op0=ALU.min, op1=ALU.add)
                nc.vector.tensor_scalar(out=v2q[:], in0=v2a[:], scalar1=MAGIC,
                                        scalar2=None, op0=ALU.subtract)

                psv3 = hp.tile([3, BL], F32, name="psv3", tag="psv")
                nc.tensor.matmul(psv3[:], hcol("vl3T"), v2q[:], start=True, stop=True)
                vout = hs.tile([3, BL], F32)
                nc.scalar.activation(vout[:], psv3[:], AF.Identity,
                                     bias=bh_("vl3b", 0, 3), scale=1.0 / 128.0)
                nc.sync.dma_start(out=value_d[:], in_=vout[:])

                # ---- policy
                psp1 = hp.tile([64, BL], F32, name="psp1", tag="psv")
                nc.tensor.matmul(psp1[:], hcol("pw1T"), pwin[:], start=True, stop=True)
                pwa = hq.tile([64, BL], F32, name="pwa", tag="v1a")
                nc.scalar.activation(pwa[:], psp1[:], AF.Relu,
                                     bias=bh_("pw1b", 0, 64), scale=1.0)
                pw1q = hq.tile([64, BL], F16, name="pw1q", tag="v1q")
                nc.vector.tensor_scalar(out=pwa[:], in0=pwa[:], scalar1=127.0,
                                        scalar2=MAGIC, op0=ALU.min, op1=ALU.add)
                nc.vector.tensor_scalar(out=pw1q[:], in0=pwa[:], scalar1=MAGIC,
                                        scalar2=None, op0=ALU.subtract)

                pwct = hs.tile([32, 16 * BL], F32)   # [c, b*16+m]
                pwcb = hs.tile([16, BL], F32)
                c0pw2, _ = hmap["pw2T"]
                for j in range(5):
                    rows = 128 if j < 4 else 16
                    pspj = hp.tile([128, BL], F32, name=f"pspj{j}", tag="st_p1")
                    nc.tensor.matmul(pspj[0:rows],
                                     whead[0:64, c0pw2 + 128 * j:c0pw2 + 128 * j + rows],
                                     pw1q[:], start=True, stop=True)
                    yj = hq.tile([128, BL], F32, name=f"yj{j}", tag="st_v1")
                    nc.scalar.activation(yj[0:rows], pspj[0:rows], AF.Identity,
                                         bias=bh_(f"pw2b{j}", 0, rows), scale=1.0)
                    if j < 4:
                        nc.vector.tensor_scalar(out=yj[0:rows], in0=yj[0:rows],
                                                scalar1=128.0, scalar2=32767.0,
                                                op0=ALU.mult, op1=ALU.min)
                        nc.vector.tensor_scalar(out=yj[0:rows], in0=yj[0:rows],
                                                scalar1=-32768.0, scalar2=None,
                                                op0=ALU.max)
                        for mr in range(4):
                            m = 4 * j + mr
                            nc.sync.dma_start(
                                out=pwct[:, m::16],
                                in_=yj[32 * mr:32 * mr + 32, :],
                            )
                    else:
                        nc.vector.tensor_scalar(out=pwcb[:], in0=yj[0:16],
                                                scalar1=16384.0, scalar2=None,
                                                op0=ALU.mult)
                if debug:
                    nc.sync.dma_start(out=dram["pwc_dbg"][:], in_=pwct[:])

                fdqv = fdq[:].rearrange("p (b f) -> p b f", f=HW)
                for b in range(BL):
                    pspol = hp.tile([16, HW], F32, name="pspol", tag="pspol", bufs=2)
                    nc.tensor.matmul(pspol[:], pwct[:, 16 * b:16 * (b + 1)],
                                     fdqv[:, b, :], start=True, stop=True)
                    polr = hq.tile([16, HW], F32, name="polr", tag="polr")
                    nc.scalar.activation(polr[:], pspol[:], AF.Relu,
                                         bias=pwcb[:, b:b + 1], scale=1.0)
                    pso = hp.tile([1, HW], F32, name="pso", tag="pso", bufs=1)
                    nc.tensor.matmul(pso[:], wheadf[0:16, 64:65], polr[:],
                                     start=True, stop=True)
                    pout = hq.tile([1, HW], F32, name="pout", tag="pout")
                    nc.scalar.activation(pout[:], pso[:], AF.Identity,
                                         bias=bh_("poutb", 0, 1), scale=1.0 / (2.0**21))
                    nc.sync.dma_start(out=policy_d[b:b + 1, :], in_=pout[:])

    split_waits(nc)
    return nc, dram


# ---------------------------------------------------------------- entry point

def _run(board_input, params, debug=False, trace=False):
    board_input = np.asarray(board_input, np.float32)
    B = board_input.shape[0]
    assert B == N_CORES * BL

    wd, tmap, bmap, hmap, bhmap = prep_weights(params)
    shapes = {k: list(v.shape) for k, v in wd.items()}

    key = ("prog", debug)
    if key not in _prog_cache:
        _prog_cache[key] = build_program(tmap, bmap, hmap, bhmap, shapes, debug=debug)
    nc, dram = _prog_cache[key]

    boards = board_input.transpose(1, 0, 2, 3).astype(np.float16)  # [2, B, 15, 15]
    boards = np.pad(boards, ((0, 0), (0, 0), (1, 1), (1, 1)))       # [2, B, 17, 17]
    in_maps = []
    for c in range(N_CORES):
        m = dict(wd)
        m["board"] = np.ascontiguousarray(
            boards[:, c * BL:(c + 1) * BL].reshape(2, BL * S))
        in_maps.append(m)

    res = run_bass_kernel_spmd(nc, in_maps, list(range(N_CORES)), trace=trace)

    value = np.zeros((B, 3), np.float32)
    policy = np.zeros((B, 1, 15, 15), np.float32)
    for c in range(N_CORES):
        r = res.results[c]
        value[c * BL:(c + 1) * BL] = r["value"].T
        policy[c * BL:(c + 1) * BL, 0] = r["policy"].reshape(BL, 15, 15)
    return (value, policy), res


def kernel(board_input, params):
    (value, policy), _ = _run(board_input, params)
    return value, policy


# revision 18
# speedup vs baseline: 1.6663x; 1.6213x over previous
"""Mix9Net forward pass on 8 Trainium2 NeuronCores (Bass/Tile).

Data parallel over batch (256 -> 32 per core). The conv trunk (4 directional
silu-resnets) runs in fp16 storage with fp32 PSUM accumulation; matmuls read
interior pixels through strided APs over zero-ring padded per-sample layouts.
The quantized heads run in exact integer arithmetic (values held at x128
integer scale); fake_quant's round() is the fp32 magic-number RNE trick and
floor() is RNE plus an is_gt fixup. Weight preprocessing (quantization,
transposes, scale folding) happens on host with jax-identical semantics.
"""
import sys

sys.path.insert(0, "/opt/trn_rl_repo")

import numpy as np

import concourse.bass as bass
import concourse.tile as tile
from concourse import mybir
from concourse.bass_utils import run_bass_kernel_spmd
import concourse.bass_utils as _bu

_orig_run_command = _bu.run_command


def _rc(argv, **kw):
    argv = list(argv)
    return _orig_run_command(argv, **kw)


_bu.run_command = _rc
from concourse.vector_clock import ScopedClock, VectorClock

F32 = mybir.dt.float32
F16 = mybir.dt.float16
AF = mybir.ActivationFunctionType
ALU = mybir.AluOpType
AX = mybir.AxisListType

MAGIC = float(np.float32(1.5 * 2.0**23))

N_CORES = 8
BL = 32                   # samples per core
S = 289                   # 17*17 padded sample
SLK = 18                  # head slack
PADW = SLK + BL * S + 20  # padded-flat width per partition
HW = 225

DIR_POS = (((1, 0), (1, 1), (1, 2)),
           ((0, 1), (1, 1), (2, 1)),
           ((0, 0), (1, 1), (2, 2)),
           ((2, 0), (1, 1), (0, 2)))

# ---------------------------------------------------------------- walrus glue


class TC(tile.TileContext):
    """This container's walrus accepts at most ONE sync wait per instruction;
    emit the kernel-tail drain as a chain of single-wait drains."""

    def _drain_and_barrier(self, tick_clock, wait_clock):
        gc = tick_clock.global_clock
        n = len(gc)
        for i in range(n):
            if gc[i] <= 0:
                continue
            vec = [0] * n
            vec[i] = gc[i]
            d = self.nc.sync.drain()
            wait_clock.add_sem_waits(d.ins, ScopedClock({None: VectorClock(vec)}))
        self.nc.all_engine_barrier()
        popped = self.nc._tile_sem_poison_stack.pop()
        assert popped is self._sem_poison
        self.nc.clear_and_free_semaphores(list(self.sems.allocated().values()))
        self.nc.all_engine_barrier()


def split_waits(nc):
    """Move excess sync waits onto same-engine NOPs placed just before the
    overloaded instruction (program order preserved -> semantics unchanged)."""
    for f in nc.m.functions:
        for bb in f.blocks:
            out = []
            for inst in bb.instructions:
                si = inst.sync_info
                if si is not None:
                    budget = max(0, min(1, 2 - len(si.on_update)))
                    if len(si.on_wait) > budget:
                        waits = list(si.on_wait)
                        nkeep = len(waits) - budget
                        extra, keep = waits[:nkeep], waits[nkeep:]
                        for i, w in enumerate(extra):
                            nop = mybir.InstNoOp(name=f"{inst.name}-wsp{i}",
                                                 engine=inst.engine)
                            nop.sync_info = mybir.SyncInfo(on_wait=[w], on_update=[])
                            out.append(nop)
                        inst.sync_info = mybir.SyncInfo(on_wait=list(keep),
                                                        on_update=list(si.on_update))
                out.append(inst)
            bb.instructions[:] = out


# ---------------------------------------------------------------- host prep

def _fq(x, scale, bits, floor=False):
    x = np.asarray(x, np.float32)
    qmin = np.float32(-(2.0 ** (bits - 1)))
    qmax = np.float32(2.0 ** (bits - 1) - 1)
    y = np.clip(x * np.float32(scale), qmin, qmax)
    q = np.floor(y) if floor else np.round(y)
    return (q.astype(np.float32) / np.float32(scale)).astype(np.float32)


def prep_weights(params):
    out = {}
    tmap = {}
    tcols = []

    def addt(name, a):
        tmap[name] = (sum(x.shape[1] for x in tcols), a.shape[1])
        tcols.append(np.ascontiguousarray(np.asarray(a, np.float32)))

    for mi, mkey in enumerate(("map1", "map2")):
        p = params[mkey]
        for bi, blk in enumerate(p["blocks"]):
            for k in range(3):
                addt(f"dcw{mi}_{bi}_{k}", np.asarray(blk["dcw"], np.float32)[k].T)
            addt(f"cw{mi}_{bi}", np.asarray(blk["cw"], np.float32).T)
        addt(f"c01_{mi}", np.asarray(p["c0_1w"], np.float32).T)
        addt(f"c02_{mi}", np.asarray(p["c0_2w"], np.float32).T)
        addt(f"fw_{mi}", np.asarray(p["fw"], np.float32).T)
    ncol = sum(a.shape[1] for a in tcols)
    wtrunk = np.zeros((128, ncol), np.float16)
    c = 0
    for a in tcols:
        wtrunk[: a.shape[0], c:c + a.shape[1]] = a.astype(np.float16)
        c += a.shape[1]
    out["wtrunk"] = wtrunk

    w18 = np.zeros((18, 4 * 128), np.float32)
    for d in range(4):
        mkey = "map1" if d < 2 else "map2"
        dw = np.asarray(params[mkey]["dw"], np.float32)  # [3, 128, 2]
        for k, (i, j) in enumerate(DIR_POS[d]):
            s = ((i - 1) + 1) * 3 + ((j - 1) + 1)
            for cc in range(2):
                w18[s * 2 + cc, d * 128:(d + 1) * 128] += dw[k, :, cc]
    out["w18t"] = w18.astype(np.float16)

    bmap = {}
    bt = np.zeros((128, 24), np.float32)
    bc = [0]

    def addb(name, v):
        bmap[name] = bc[0]
        v = np.asarray(v, np.float32)
        bt[: len(v), bc[0]] = v
        bc[0] += 1

    for mi, mkey in enumerate(("map1", "map2")):
        p = params[mkey]
        addb(f"db{mi}", p["db"])
        for bi, blk in enumerate(p["blocks"]):
            addb(f"dcb{mi}_{bi}", blk["dcb"])
            addb(f"cb{mi}_{bi}", blk["cb"])
        addb(f"c01b{mi}", p["c0_1b"])
        addb(f"c02b{mi}", p["c0_2b"])
        addb(f"fb{mi}", p["fb"])
    out["btrunk"] = bt

    wq = _fq(params["dwconv_w"], 65536, 16)
    wdw = np.zeros((64, 288), np.float16)
    for i in range(3):
        for j in range(3):
            k = 3 * i + j
            wdw[32:64, 32 * k:32 * k + 32] = (np.diag(wq[:, 0, i, j]) / 32.0
                                              ).astype(np.float16)
    out["wdw"] = wdw

    hmap = {}
    hcols = []

    def addh(name, a):
        hmap[name] = (sum(x.shape[1] for x in hcols), a.shape[1])
        hcols.append(np.asarray(a, np.float32))

    addh("pw1T", _fq(params["pw1_w"], 128, 8).T)
    addh("pw2T", _fq(params["pw2_w"], 128, 8).T)
    for sk in ("vc", "ve", "vm", "vq"):
        sp = params[sk]
        addh(f"{sk}_u1T", _fq(sp["u1w"], 128, 8).T)
        addh(f"{sk}_u2T", _fq(sp["u2w"], 128, 8).T)
        addh(f"{sk}_dwT", _fq(sp["dw"], 128, 8).T)
    vl1 = _fq(params["vl1_w"], 128, 8).T
    addh("vl1T0", vl1[0:128])
    addh("vl1T1", vl1[128:256])
    addh("vl1T2", vl1[256:320])
    addh("vl2T", _fq(params["vl2_w"], 128, 8).T)
    addh("vl3T", _fq(params["vl3_w"], 128, 8).T)
    ncol = sum(a.shape[1] for a in hcols)
    whead = np.zeros((128, ncol), np.float16)
    c = 0
    for a in hcols:
        whead[: a.shape[0], c:c + a.shape[1]] = a.astype(np.float16)
        c += a.shape[1]
    out["whead"] = whead

    whf = np.zeros((128, 65), np.float32)
    for j in range(64):
        whf[2 * j, j] = 1.0
        whf[2 * j + 1, j] = 1.0
    whf[0:16, 64] = np.asarray(params["pout_w"], np.float32)[0]
    out["wheadf"] = whf

    bhmap = {}
    bh = np.zeros((128, 24), np.float32)
    hc = [0]

    def addbh(name, v, dup64=False):
        bhmap[name] = hc[0]
        v = np.asarray(v, np.float32).reshape(-1)
        bh[: len(v), hc[0]] = v
        if dup64:
            bh[64:64 + len(v), hc[0]] = v
        hc[0] += 1

    for sk in ("vc", "ve", "vm", "vq"):
        sp = params[sk]
        addbh(f"{sk}_u1b", _fq(sp["u1b"], 128 * 128, 32) * 128)
        addbh(f"{sk}_u2b", _fq(sp["u2b"], 128 * 128, 32) * 128)
        addbh(f"{sk}_dwb", _fq(sp["db"], 128 * 128, 32) * 128, dup64=True)
    addbh("pw1b", _fq(params["pw1_b"], 128 * 128, 32) * 128)
    addbh("vl1b", _fq(params["vl1_b"], 128 * 128, 32) * 128)
    addbh("vl2b", _fq(params["vl2_b"], 128 * 128, 32) * 128)
    addbh("vl3b", _fq(params["vl3_b"], 128 * 128, 32))
    addbh("poutb", params["pout_b"])
    addbh("dwb", _fq(params["dwconv_b"], 128, 16) * 128)
    pw2b = _fq(params["pw2_b"], 128 * 128, 32) * 128
    for j in range(5):
        addbh(f"pw2b{j}", pw2b[128 * j:min(528, 128 * (j + 1))])
    out["bhead"] = bh

    return out, tmap, bmap, hmap, bhmap


# ---------------------------------------------------------------- device build

_prog_cache = {}


def build_program(tmap, bmap, hmap, bhmap, shapes, debug=False):
    nc = bass.Bass()
    dram = {}

    def din(name, shape, dt):
        dram[name] = nc.dram_tensor(name, list(shape), dt, kind="ExternalInput")
        return dram[name]

    def dout(name, shape, dt=F32):
        dram[name] = nc.dram_tensor(name, list(shape), dt, kind="ExternalOutput")
        return dram[name]

    board_d = din("board", [2, BL * S], F16)  # host-padded 17x17
    w18_d = din("w18t", shapes["w18t"], F16)
    wtrunk_d = din("wtrunk", shapes["wtrunk"], F16)
    btrunk_d = din("btrunk", shapes["btrunk"], F32)
    wdw_d = din("wdw", shapes["wdw"], F16)
    whead_d = din("whead", shapes["whead"], F16)
    wheadf_d = din("wheadf", shapes["wheadf"], F32)
    bhead_d = din("bhead", shapes["bhead"], F32)
    value_d = dout("value", [3, BL])
    policy_d = dout("policy", [BL, HW])
    if debug:
        dout("facc_dbg", [64, 7200])
        dout("fdq_dbg", [32, 7200])
        dout("fsum_dbg", [64, BL])
        dout("pwc_dbg", [32, 16 * BL])

    with TC(nc) as tc:
        with tc.tile_pool(name="wpool", bufs=1) as wpool:
            # ---- persistent tiles
            GW = SLK + 8 * S + 20
            xAg = [wpool.tile([128, GW], F16, name=f"xAg{i}") for i in range(4)]
            xBg = [wpool.tile([128, GW], F16, name=f"xBg{i}") for i in range(4)]
            X18 = wpool.tile([18, PADW], F16)
            PAD1 = wpool.tile([128, PADW], F16)  # r0-1 board, r32-63 fdin, r64-127 frel
            faccg = [wpool.tile([64, 1800], F32, name=f"faccg{i}") for i in range(4)]
            fdq = wpool.tile([32, 7200], F32)
            wtrunk = wpool.tile([128, shapes["wtrunk"][1]], F16)
            w18t = wpool.tile([18, shapes["w18t"][1]], F16)
            btrunk = wpool.tile([128, shapes["btrunk"][1]], F32)
            wdw = wpool.tile([64, shapes["wdw"][1]], F16)
            whead = wpool.tile([128, shapes["whead"][1]], F16)
            wheadf = wpool.tile([128, shapes["wheadf"][1]], F32)
            bhead = wpool.tile([128, shapes["bhead"][1]], F32)

            for i in range(4):
                nc.vector.memset(xAg[i][:], 0.0)
                nc.gpsimd.memset(xBg[i][:], 0.0)
            nc.vector.memset(PAD1[:], 0.0)

            nc.gpsimd.dma_start(out=wtrunk[:], in_=wtrunk_d[:])
            nc.gpsimd.dma_start(out=w18t[:], in_=w18_d[:])
            nc.gpsimd.dma_start(out=btrunk[:], in_=btrunk_d[:])
            nc.gpsimd.dma_start(out=wdw[:], in_=wdw_d[:])
            nc.gpsimd.dma_start(out=whead[:], in_=whead_d[:])
            nc.gpsimd.dma_start(out=wheadf[:], in_=wheadf_d[:])
            nc.gpsimd.dma_start(out=bhead[:], in_=bhead_d[:])

            def padview(t):  # [128, BL, 17, 17] (full-width tiles)
                return t[:, SLK:SLK + BL * S].rearrange(
                    "p (b h w) -> p b h w", h=17, w=17)

            def gview(t):  # [128, 8, 17, 17] (group tiles)
                return t[:, SLK:SLK + 8 * S].rearrange(
                    "p (b h w) -> p b h w", h=17, w=17)

            def ginterior(t, b0, nb):  # samples b0..b0+nb within the group
                return gview(t)[:, b0:b0 + nb, 1:16, 1:16]

            def ginterior5(t):  # [*, 4, 2, 15, 15] whole group
                v = t[:, SLK:SLK + 8 * S].rearrange(
                    "p (a b h w) -> p a b h w", a=4, b=2, h=17, w=17)
                return v[:, :, :, 1:16, 1:16]

            pv1 = padview(PAD1)
            nc.gpsimd.dma_start(out=PAD1[0:2, SLK:SLK + BL * S], in_=board_d[:])

            for s in range(9):
                di, dj = s // 3 - 1, s % 3 - 1
                sh = di * 17 + dj
                nc.sync.dma_start(
                    out=X18[2 * s:2 * s + 2, SLK:SLK + BL * S],
                    in_=PAD1[0:2, SLK + sh:SLK + sh + BL * S],
                )
            x18v = padview(X18)

            def tb(name):
                return btrunk[:, bmap[name]:bmap[name] + 1]

            def wcol(name):
                c0, w = tmap[name]
                return wtrunk[:, c0:c0 + w]

            with tc.tile_pool(name="ppool", bufs=2, space="PSUM") as ppool:
                # ================= trunk =================
                for d in range(4):
                    mi = d // 2
                    shifts = [((i - 1), (j - 1)) for (i, j) in DIR_POS[d]]
                    curg, nxtg = xAg, xBg

                    for g in range(4):
                        ps = ppool.tile([128, 4, 512], F32, name="ps1", tag="ps")
                        for pr in range(4):
                            b0 = 8 * g + 2 * pr
                            nc.tensor.matmul(
                                ps[:, pr, 0:450],
                                w18t[:, d * 128:(d + 1) * 128],
                                x18v[:, b0:b0 + 2, 1:16, 1:16],
                                start=True, stop=True,
                            )
                        nc.scalar.activation(
                            ginterior5(curg[g]),
                            ps[:, :, 0:450].rearrange(
                                "p a (b h w) -> p a b h w", b=2, h=15, w=15),
                            AF.Silu, bias=tb(f"db{mi}"), scale=1.0,
                        )

                    for bi in range(4):
                        for g in range(4):
                            ps = ppool.tile([128, 4, 512], F32, name="psd", tag="ps")
                            for pr in range(4):
                                for k in range(3):
                                    di, dj = shifts[k]
                                    rhs = gview(curg[g])[:, 2 * pr:2 * pr + 2,
                                                         1 + di:16 + di,
                                                         1 + dj:16 + dj]
                                    nc.tensor.matmul(
                                        ps[:, pr, 0:450],
                                        wcol(f"dcw{mi}_{bi}_{k}"),
                                        rhs,
                                        start=(k == 0), stop=(k == 2),
                                    )
                            tg = wpool.tile([128, 1800], F16, name="tg",
                                            tag="tg", bufs=2)
                            nc.scalar.activation(
                                tg[:].rearrange("p (a b) -> p a b", a=4),
                                ps[:, :, 0:450],
                                AF.Silu, bias=tb(f"dcb{mi}_{bi}"), scale=1.0,
                            )
                            ps2 = ppool.tile([128, 4, 512], F32, name="psc", tag="ps")
                            for pr in range(4):
                                nc.tensor.matmul(
                                    ps2[:, pr, 0:450],
                                    wcol(f"cw{mi}_{bi}"),
                                    tg[:, 450 * pr:450 * (pr + 1)],
                                    start=True, stop=True,
                                )
                            nc.scalar.activation(
                                ginterior5(nxtg[g]),
                                ps2[:, :, 0:450].rearrange(
                                    "p a (b h w) -> p a b h w", b=2, h=15, w=15),
                                AF.Silu, bias=tb(f"cb{mi}_{bi}"), scale=1.0,
                            )
                            nc.vector.tensor_tensor(out=nxtg[g][:], in0=nxtg[g][:],
                                                    in1=curg[g][:], op=ALU.add)
                        curg, nxtg = nxtg, curg

                    for g in range(4):
                        ps = ppool.tile([128, 4, 512], F32, name="psu", tag="ps")
                        for pr in range(4):
                            nc.tensor.matmul(
                                ps[:, pr, 0:450], wcol(f"c01_{mi}"),
                                ginterior(curg[g], 2 * pr, 2), start=True, stop=True,
                            )
                        u1 = wpool.tile([128, 1800], F16, name="u1", tag="tg", bufs=2)
                        nc.scalar.activation(
                            u1[:].rearrange("p (a b) -> p a b", a=4),
                            ps[:, :, 0:450],
                            AF.Silu, bias=tb(f"c01b{mi}"), scale=1.0,
                        )
                        ps2 = ppool.tile([128, 4, 512], F32, name="psu2", tag="ps")
                        for pr in range(4):
                            nc.tensor.matmul(
                                ps2[:, pr, 0:450], wcol(f"c02_{mi}"),
                                u1[:, 450 * pr:450 * (pr + 1)], start=True, stop=True,
                            )
                        nc.scalar.activation(
                            ginterior5(nxtg[g]),
                            ps2[:, :, 0:450].rearrange(
                                "p a (b h w) -> p a b h w", b=2, h=15, w=15),
                            AF.Silu, bias=tb(f"c02b{mi}"), scale=1.0,
                        )
                        nc.vector.tensor_tensor(out=nxtg[g][:], in0=nxtg[g][:],
                                                in1=curg[g][:], op=ALU.add)
                        psF = ppool.tile([64, 4, 512], F32, name="psf", tag="ps")
                        for pr in range(4):
                            nc.tensor.matmul(
                                psF[:, pr, 0:450], wcol(f"fw_{mi}"),
                                ginterior(nxtg[g], 2 * pr, 2), start=True, stop=True,
                            )
                        z = wpool.tile([64, 1800], F32, name="z", tag="z", bufs=1)
                        zv = z[:].rearrange("p (a b) -> p a b", a=4)
                        nc.vector.tensor_scalar(
                            out=zv, in0=psF[:, :, 0:450],
                            scalar1=btrunk[0:64, bmap[f"fb{mi}"]:bmap[f"fb{mi}"] + 1],
                            scalar2=32.0, op0=ALU.add, op1=ALU.mult,
                        )
                        nc.vector.tensor_scalar(
                            out=z[:], in0=z[:], scalar1=512.0, scalar2=-512.0,
                            op0=ALU.min, op1=ALU.max,
                        )
                        if d == 0:
                            nc.vector.tensor_scalar(
                                out=faccg[g][:], in0=z[:], scalar1=MAGIC,
                                scalar2=MAGIC, op0=ALU.add, op1=ALU.subtract,
                            )
                        else:
                            nc.vector.tensor_scalar(
                                out=z[:], in0=z[:], scalar1=MAGIC, scalar2=MAGIC,
                                op0=ALU.add, op1=ALU.subtract,
                            )
                            nc.vector.tensor_tensor(out=faccg[g][:], in0=z[:],
                                                    in1=faccg[g][:], op=ALU.add)

                # ============== feature stage ==============
                for g in range(4):
                    nc.scalar.activation(PAD1[64:128, 1800 * g:1800 * (g + 1)],
                                         faccg[g][:], AF.Relu, bias=0.0, scale=1.0)
                    nc.scalar.activation(
                        gview(PAD1)[32:64, :, 1:16, 1:16] if False else
                        pv1[32:64, 8 * g:8 * g + 8, 1:16, 1:16],
                        faccg[g][0:32, :].rearrange("p (b h w) -> p b h w",
                                                    h=15, w=15),
                        AF.Relu, bias=0.0, scale=1.0,
                    )

                for g in range(4):
                    psdw = ppool.tile([32, 4, 512], F32, name="psdw", tag="ps")
                    for pr in range(4):
                        b0 = 8 * g + 2 * pr
                        for k in range(9):
                            di, dj = k // 3 - 1, k % 3 - 1
                            rhs = pv1[32:64, b0:b0 + 2,
                                      1 + di:16 + di, 1 + dj:16 + dj]
                            nc.tensor.matmul(
                                psdw[:, pr, 0:450],
                                wdw[32:64, 32 * k:32 * k + 32],
                                rhs,
                                start=(k == 0), stop=(k == 8),
                                tile_position=(32, 0),
                            )
                    fdt = wpool.tile([32, 1800], F32, name="fdt", tag="z", bufs=1)
                    nc.scalar.activation(
                        fdt[:].rearrange("p (a b) -> p a b", a=4),
                        psdw[:, :, 0:450], AF.Relu,
                        bias=bhead[0:32, bhmap["dwb"]:bhmap["dwb"] + 1], scale=128.0,
                    )
                    nc.vector.tensor_scalar(
                        out=fdt[:], in0=fdt[:], scalar1=32767.0, scalar2=MAGIC,
                        op0=ALU.min, op1=ALU.add,
                    )
                    nc.vector.tensor_scalar(
                        out=fdq[:, 1800 * g:1800 * (g + 1)], in0=fdt[:],
                        scalar1=MAGIC, scalar2=None, op0=ALU.subtract,
                    )

            if debug:
                for g in range(4):
                    nc.sync.dma_start(out=dram["facc_dbg"][:, 1800 * g:1800 * (g + 1)],
                                      in_=faccg[g][:])
                nc.sync.dma_start(out=dram["fdq_dbg"][:], in_=fdq[:])

            # ================= heads =================
            with (
                tc.tile_pool(name="hq", bufs=4) as hq,
                tc.tile_pool(name="hs", bufs=1) as hs,
                tc.tile_pool(name="hp", bufs=1, space="PSUM") as hp,
            ):
                frel = PAD1[64:128, 0:7200]

                # two-pass band sums -> 9 region sums + full sum, batched
                # colband[c, b, h, jb] = sum_w-in-band ; then rows, then total
                cb_fd = hs.tile([128, BL * 15 * 3], F32)
                nc.vector.tensor_reduce(
                    out=cb_fd[0:32, :],
                    in_=fdq[:].rearrange("p (b h jb w) -> p b h jb w", h=15, jb=3, w=5),
                    axis=AX.X, op=ALU.add,
                )
                nc.vector.tensor_reduce(
                    out=cb_fd[96:128, :],
                    in_=frel[32:64, :].rearrange("p (b h jb w) -> p b h jb w",
                                                 h=15, jb=3, w=5),
                    axis=AX.X, op=ALU.add,
                )
                # rsum[c, b, ib, jb] = sum_h-in-band colband
                rsum = hs.tile([128, 10, BL], F32)  # cols 0..8 regions (ib*3+jb), 9=fsum
                cbv = cb_fd[:].rearrange("p (b ib hr jb) -> p b ib hr jb",
                                         ib=3, hr=5, jb=3)
                for ib in range(3):
                    for jb in range(3):
                        nc.vector.tensor_reduce(
                            out=rsum[0:32, ib * 3 + jb, :],
                            in_=cbv[0:32, :, ib, :, jb],
                            axis=AX.X, op=ALU.add,
                        )
                        nc.vector.tensor_reduce(
                            out=rsum[96:128, ib * 3 + jb, :],
                            in_=cbv[96:128, :, ib, :, jb],
                            axis=AX.X, op=ALU.add,
                        )
                nc.vector.tensor_reduce(
                    out=rsum[0:32, 9, :],
                    in_=rsum[0:32, 0:9, :].rearrange("p a b -> p b a"),
                    axis=AX.X, op=ALU.add,
                )
                nc.vector.tensor_reduce(
                    out=rsum[96:128, 9, :],
                    in_=rsum[96:128, 0:9, :].rearrange("p a b -> p b a"),
                    axis=AX.X, op=ALU.add,
                )
                nc.sync.dma_start(out=rsum[32:64, :, :], in_=rsum[96:128, :, :])

                def floor_chain(src_ap, shape, pre_mult, name):
                    """floor(src*pre_mult) elementwise -> f32 tile [64, *shape]"""
                    y = hq.tile([64] + shape, F32, name=f"{name}_y", tag=f"{name}_y")
                    nc.vector.tensor_scalar(out=y[:], in0=src_ap, scalar1=pre_mult,
                                            scalar2=None, op0=ALU.mult)
                    r = hq.tile([64] + shape, F32, name=f"{name}_r", tag=f"{name}_r")
                    nc.vector.tensor_scalar(out=r[:], in0=y[:], scalar1=MAGIC,
                                            scalar2=MAGIC, op0=ALU.add,
                                            op1=ALU.subtract)
                    gt = hq.tile([64] + shape, F32, name=f"{name}_g", tag=f"{name}_g")
                    nc.vector.tensor_tensor(out=gt[:], in0=r[:], in1=y[:],
                                            op=ALU.is_gt)
                    nc.vector.tensor_tensor(out=r[:], in0=r[:], in1=gt[:],
                                            op=ALU.subtract)
                    return r

                fsum_i = floor_chain(rsum[0:64, 9, :], [BL], 1.0 / 256.0, "fsum")
                if debug:
                    nc.sync.dma_start(out=dram["fsum_dbg"][:], in_=fsum_i[:])

                pwin = hs.tile([64, BL], F16)
                nc.vector.tensor_scalar(out=pwin[:], in0=fsum_i[:], scalar1=127.0,
                                        scalar2=-128.0, op0=ALU.min, op1=ALU.max)

                reg_i = floor_chain(rsum[0:64, 0:9, :], [9, BL], 1.0 / 32.0, "regs")
                regq9 = hs.tile([64, 9, BL], F16)
                nc.vector.tensor_scalar(out=regq9[:], in0=reg_i[:], scalar1=127.0,
                                        scalar2=-128.0, op0=ALU.min, op1=ALU.max)
                regq = {(i, j): regq9[:, i * 3 + j, :] for i in range(3)
                        for j in range(3)}

                HROWS = {"pw1T": 64, "pw2T": 64, "vl1T0": 128, "vl1T1": 128,
                         "vl1T2": 64, "vl2T": 64, "vl3T": 64}

                def hcol(name):
                    c0, w = hmap[name]
                    rows = HROWS.get(name, 64)
                    return whead[0:rows, c0:c0 + w]

                def bh_(name, r0, rows):
                    return bhead[r0:r0 + rows, bhmap[name]:bhmap[name] + 1]

                def quant8_floor(v_ap, rows, relu, name):
                    c = hq.tile([128, BL], F32, name=f"{name}_c", tag="q8_c")
                    if relu:
                        nc.vector.tensor_scalar(out=c[0:rows], in0=v_ap,
                                                scalar1=127.0, scalar2=None,
                                                op0=ALU.min)
                    else:
                        nc.vector.tensor_scalar(out=c[0:rows], in0=v_ap,
                                                scalar1=127.0, scalar2=-128.0,
                                                op0=ALU.min, op1=ALU.max)
                    r = hq.tile([128, BL], F32, name=f"{name}_r", tag="q8_r")
                    nc.vector.tensor_scalar(out=r[0:rows], in0=c[0:rows],
                                            scalar1=MAGIC, scalar2=MAGIC,
                                            op0=ALU.add, op1=ALU.subtract)
                    g = hq.tile([128, BL], F32, name=f"{name}_g", tag="q8_g")
                    nc.vector.tensor_tensor(out=g[0:rows], in0=r[0:rows],
                                            in1=c[0:rows], op=ALU.is_gt)
                    q = hq.tile([128, BL], F16, name=f"{name}_q", tag="q8_q")
                    nc.vector.tensor_tensor(out=q[0:rows], in0=r[0:rows],
                                            in1=g[0:rows], op=ALU.subtract)
                    return q

                def star(xq, pset, col_off, name):
                    ps1 = hp.tile([128, BL], F32, name=f"{name}_p1", tag="st_p1")
                    nc.tensor.matmul(ps1[:], hcol(f"{pset}_u1T"), xq[0:64],
                                     start=True, stop=True)
                    v1 = hq.tile([128, BL], F32, name=f"{name}_v1", tag="st_v1")
                    nc.scalar.activation(v1[:], ps1[:], AF.Relu,
                                         bias=bh_(f"{pset}_u1b", 0, 128), scale=1.0)
                    x1q = quant8_floor(v1[:], 128, True, f"{name}_x1")
                    ps2 = hp.tile([128, BL], F32, name=f"{name}_p2", tag="st_p2")
                    nc.tensor.matmul(ps2[:], hcol(f"{pset}_u2T"), xq[0:64],
                                     start=True, stop=True)
                    v2 = hq.tile([128, BL], F32, name=f"{name}_v2", tag="st_v2")
                    nc.scalar.activation(v2[:], ps2[:], AF.Identity,
                                         bias=bh_(f"{pset}_u2b", 0, 128), scale=1.0)
                    x2q = quant8_floor(v2[:], 128, False, f"{name}_x2")
                    p = hq.tile([128, BL], F32, name=f"{name}_pp", tag="st_pp")
                    nc.vector.tensor_tensor(out=p[:], in0=x1q[:], in1=x2q[:],
                                            op=ALU.mult)
                    ps3 = hp.tile([64, BL], F32, name=f"{name}_p3", tag="st_p34")
                    nc.tensor.matmul(ps3[:], wheadf[:, 0:64], p[:],
                                     start=True, stop=True)
                    y3 = hq.tile([64, BL], F32, name=f"{name}_y3", tag="st_y3")
                    nc.vector.tensor_scalar(out=y3[:], in0=ps3[:],
                                            scalar1=1.0 / 128.0, scalar2=127.0,
                                            op0=ALU.mult, op1=ALU.min)
                    nc.vector.tensor_scalar(out=y3[:], in0=y3[:], scalar1=-128.0,
                                            scalar2=None, op0=ALU.max)
                    r3 = hq.tile([64, BL], F32, name=f"{name}_r3", tag="st_r3")
                    nc.vector.tensor_scalar(out=r3[:], in0=y3[:], scalar1=MAGIC,
                                            scalar2=MAGIC, op0=ALU.add,
                                            op1=ALU.subtract)
                    g3 = hq.tile([64, BL], F32, name=f"{name}_g3", tag="st_g3")
                    nc.vector.tensor_tensor(out=g3[:], in0=r3[:], in1=y3[:],
                                            op=ALU.is_gt)
                    x3q = hq.tile([64, BL], F16, name=f"{name}_x3", tag="st_x3")
                    nc.vector.tensor_tensor(out=x3q[:], in0=r3[:], in1=g3[:],
                                            op=ALU.subtract)
                    ps4 = hp.tile([128, BL], F32, name=f"{name}_p4", tag="st_p34")
                    sl = slice(col_off, col_off + 64)
                    nc.tensor.matmul(ps4[sl], hcol(f"{pset}_dwT"), x3q[:],
                                     start=True, stop=True,
                                     tile_position=(0, col_off))
                    so = hq.tile([128, BL], F32, name=f"{name}_so", tag="st_so",
                                 bufs=10)
                    nc.scalar.activation(so[sl], ps4[sl], AF.Relu,
                                         bias=bh_(f"{pset}_dwb", col_off, 64),
                                         scale=1.0)
                    return so

                vmat = {}
                for (i, j), pset in (((0, 0), "vc"), ((0, 2), "vc"), ((2, 0), "vc"),
                                     ((2, 2), "vc"), ((0, 1), "ve"), ((1, 0), "ve"),
                                     ((1, 2), "ve"), ((2, 1), "ve"), ((1, 1), "vm")):
                    vmat[(i, j)] = star(regq[(i, j)], pset, 0, f"v{i}{j}")

                def avg2(a_ap, b_ap, name):
                    s = hq.tile([64, BL], F32, name=f"{name}_s", tag="av_s")
                    nc.vector.tensor_tensor(out=s[:], in0=a_ap, in1=b_ap, op=ALU.add)
                    y = hq.tile([64, BL], F32, name=f"{name}_y", tag="av_y")
                    nc.vector.tensor_scalar(out=y[:], in0=s[:], scalar1=0.5,
                                            scalar2=127.0, op0=ALU.mult, op1=ALU.min)
                    nc.vector.tensor_scalar(out=y[:], in0=y[:], scalar1=-128.0,
                                            scalar2=None, op0=ALU.max)
                    r = hq.tile([64, BL], F32, name=f"{name}_r", tag="av_r")
                    nc.vector.tensor_scalar(out=r[:], in0=y[:], scalar1=MAGIC,
                                            scalar2=MAGIC, op0=ALU.add,
                                            op1=ALU.subtract)
                    g = hq.tile([64, BL], F32, name=f"{name}_g", tag="av_g")
                    nc.vector.tensor_tensor(out=g[:], in0=r[:], in1=y[:],
                                            op=ALU.is_gt)
                    q = hq.tile([64, BL], F32, name=f"{name}_q", tag="av_q")
                    nc.vector.tensor_tensor(out=q[:], in0=r[:], in1=g[:],
                                            op=ALU.subtract)
                    return q

                def avg4(i0, i1, i2, i3, name):
                    ab = avg2(vmat[i0][0:64], vmat[i1][0:64], f"{name}ab")
                    cd = avg2(vmat[i2][0:64], vmat[i3][0:64], f"{name}cd")
                    fq = avg2(ab[:], cd[:], f"{name}f")
                    q16 = hq.tile([64, BL], F16, name=f"{name}_q16", tag="av_q16")
                    nc.vector.tensor_copy(out=q16[:], in_=fq[:])
                    return q16

                a00 = avg4((0, 0), (0, 1), (1, 0), (1, 1), "a00")
                a01 = avg4((0, 1), (0, 2), (1, 1), (1, 2), "a01")
                a10 = avg4((1, 0), (1, 1), (2, 0), (2, 1), "a10")
                a11 = avg4((1, 1), (1, 2), (2, 1), (2, 2), "a11")

                q00 = star(a00, "vq", 64, "q00")
                q01 = star(a01, "vq", 0, "q01")
                q10 = star(a10, "vq", 64, "q10")
                q11 = star(a11, "vq", 0, "q11")

                vcat = hs.tile([128, 3, BL], F16)
                nc.vector.tensor_copy(out=vcat[0:64, 0, :], in_=pwin[:])
                for qv, r0, ci in ((q00, 64, 0), (q01, 0, 1),
                                   (q10, 64, 1), (q11, 0, 2)):
                    t1 = hq.tile([128, BL], F32, name="vqt", tag="vq_t")
                    sl = slice(r0, r0 + 64)
                    nc.vector.tensor_scalar(out=t1[sl], in0=qv[sl], scalar1=127.0,
                                            scalar2=MAGIC, op0=ALU.min, op1=ALU.add)
                    nc.vector.tensor_scalar(out=vcat[sl, ci, :], in0=t1[sl],
                                            scalar1=MAGIC, scalar2=None,
                                            op0=ALU.subtract)

                psv = hp.tile([64, BL], F32, name="psv", tag="psv")
                nc.tensor.matmul(psv[:], hcol("vl1T0"), vcat[:, 0, :],
                                 start=True, stop=False)
                nc.tensor.matmul(psv[:], hcol("vl1T1"), vcat[:, 1, :],
                                 start=False, stop=False)
                nc.tensor.matmul(psv[:], hcol("vl1T2"), vcat[0:64, 2, :],
                                 start=False, stop=True)
                v1a = hq.tile([64, BL], F32, name="v1a", tag="v1a")
                nc.scalar.activation(v1a[:], psv[:], AF.Relu,
                                     bias=bh_("vl1b", 0, 64), scale=1.0)
                v1q = hq.tile([64, BL], F16, name="v1q", tag="v1q")
                nc.vector.tensor_scalar(out=v1a[:], in0=v1a[:], scalar1=127.0,
                                        scalar2=MAGIC, op0=ALU.min, op1=ALU.add)
                nc.vector.tensor_scalar(out=v1q[:], in0=v1a[:], scalar1=MAGIC,
                                        scalar2=None, op0=ALU.subtract)

                psv2 = hp.tile([64, BL], F32, name="psv2", tag="psv")
                nc.tensor.matmul(psv2[:], hcol("vl2T"), v1q[:], start=True, stop=True)
                v2a = hq.tile([64, BL], F32, name="v2a", tag="v1a")
                nc.scalar.activation(v2a[:], psv2[:], AF.Relu,
                                     bias=bh_("vl2b", 0, 64), scale=1.0)
                v2q = hq.tile([64, BL], F16, name="v2q", tag="v1q")
                nc.vector.tensor_scalar(out=v2a[:], in0=v2a[:], scalar1=127.0,
                                        scalar2=MAGIC, op0=ALU.min, op1=ALU.add)
                nc.vector.tensor_scalar(out=v2q[:], in0=v2a[:], scalar1=MAGIC,
                                        scalar2=None, op0=ALU.subtract)

                psv3 = hp.tile([3, BL], F32, name="psv3", tag="psv")
                nc.tensor.matmul(psv3[:], hcol("vl3T"), v2q[:], start=True, stop=True)
                vout = hs.tile([3, BL], F32)
                nc.scalar.activation(vout[:], psv3[:], AF.Identity,
                                     bias=bh_("vl3b", 0, 3), scale=1.0 / 128.0)
                nc.sync.dma_start(out=value_d[:], in_=vout[:])

                # ---- policy
                psp1 = hp.tile([64, BL], F32, name="psp1", tag="psv")
                nc.tensor.matmul(psp1[:], hcol("pw1T"), pwin[:], start=True, stop=True)
                pwa = hq.tile([64, BL], F32, name="pwa", tag="v1a")
                nc.scalar.activation(pwa[:], psp1[:], AF.Relu,
                                     bias=bh_("pw1b", 0, 64), scale=1.0)
                pw1q = hq.tile([64, BL], F16, name="pw1q", tag="v1q")
                nc.vector.tensor_scalar(out=pwa[:], in0=pwa[:], scalar1=127.0,
                                        scalar2=MAGIC, op0=ALU.min, op1=ALU.add)
                nc.vector.tensor_scalar(out=pw1q[:], in0=pwa[:], scalar1=MAGIC,
                                        scalar2=None, op0=ALU.subtract)

                pwct = hs.tile([32, 16 * BL], F32)   # [c, b*16+m]
                pwcb = hs.tile([16, BL], F32)
                c0pw2, _ = hmap["pw2T"]
                for j in range(5):
                    rows = 128 if j < 4 else 16
                    pspj = hp.tile([128, BL], F32, name=f"pspj{j}", tag="st_p1")
                    nc.tensor.matmul(pspj[0:rows],
                                     whead[0:64, c0pw2 + 128 * j:c0pw2 + 128 * j + rows],
                                     pw1q[:], start=True, stop=True)
                    yj = hq.tile([128, BL], F32, name=f"yj{j}", tag="st_v1")
                    nc.scalar.activation(yj[0:rows], pspj[0:rows], AF.Identity,
                                         bias=bh_(f"pw2b{j}", 0, rows), scale=1.0)
                    if j < 4:
                        nc.vector.tensor_scalar(out=yj[0:rows], in0=yj[0:rows],
                                                scalar1=128.0, scalar2=32767.0,
                                                op0=ALU.mult, op1=ALU.min)
                        nc.vector.tensor_scalar(out=yj[0:rows], in0=yj[0:rows],
                                                scalar1=-32768.0, scalar2=None,
                                                op0=ALU.max)
                        for mr in range(4):
                            m = 4 * j + mr
                            nc.sync.dma_start(
                                out=pwct[:, m::16],
                                in_=yj[32 * mr:32 * mr + 32, :],
                            )
                    else:
                        nc.vector.tensor_scalar(out=pwcb[:], in0=yj[0:16],
                                                scalar1=16384.0, scalar2=None,
                                                op0=ALU.mult)
                if debug:
                    nc.sync.dma_start(out=dram["pwc_dbg"][:], in_=pwct[:])

                fdqv = fdq[:].rearrange("p (b f) -> p b f", f=HW)
                for j8 in range(4):  # blocks of 8 samples
                    polr8 = wpool.tile([16, 8 * HW], F32, name="polr8",
                                       tag="z", bufs=1)
                    for br in range(8):
                        b = 8 * j8 + br
                        pspol = hp.tile([16, HW], F32, name="pspol",
                                        tag="pspol", bufs=2)
                        nc.tensor.matmul(pspol[:], pwct[:, 16 * b:16 * (b + 1)],
                                         fdqv[:, b, :], start=True, stop=True)
                        nc.scalar.activation(polr8[:, HW * br:HW * (br + 1)],
                                             pspol[:], AF.Relu,
                                             bias=pwcb[:, b:b + 1], scale=1.0)
                    for c4 in range(4):
                        pso = hp.tile([1, 450], F32, name="pso", tag="pso", bufs=2)
                        nc.tensor.matmul(pso[:], wheadf[0:16, 64:65],
                                         polr8[:, 450 * c4:450 * (c4 + 1)],
                                         start=True, stop=True)
                        pout = hq.tile([1, 450], F32, name="pout", tag="pout")
                        nc.vector.tensor_scalar(
                            out=pout[:], in0=pso[:],
                            scalar1=1.0 / (2.0**21), scalar2=bh_("poutb", 0, 1),
                            op0=ALU.mult, op1=ALU.add,
                        )
                        nc.sync.dma_start(
                            out=policy_d[8 * j8 + 2 * c4:8 * j8 + 2 * c4 + 2, :],
                            in_=pout[:],
                        )

    split_waits(nc)
    return nc, dram


# ---------------------------------------------------------------- entry point

def _run(board_input, params, debug=False, trace=False):
    board_input = np.asarray(board_input, np.float32)
    B = board_input.shape[0]
    assert B == N_CORES * BL

    wd, tmap, bmap, hmap, bhmap = prep_weights(params)
    shapes = {k: list(v.shape) for k, v in wd.items()}

    key = ("prog", debug)
    if key not in _prog_cache:
        _prog_cache[key] = build_program(tmap, bmap, hmap, bhmap, shapes, debug=debug)
    nc, dram = _prog_cache[key]

    boards = board_input.transpose(1, 0, 2, 3).astype(np.float16)  # [2, B, 15, 15]
    boards = np.pad(boards, ((0, 0), (0, 0), (1, 1), (1, 1)))       # [2, B, 17, 17]
    in_maps = []
    for c in range(N_CORES):
        m = dict(wd)
        m["board"] = np.ascontiguousarray(
            boards[:, c * BL:(c + 1) * BL].reshape(2, BL * S))
        in_maps.append(m)

    res = run_bass_kernel_spmd(nc, in_maps, list(range(N_CORES)), trace=trace)

    value = np.zeros((B, 3), np.float32)
    policy = np.zeros((B, 1, 15, 15), np.float32)
    for c in range(N_CORES):
        r = res.results[c]
        value[c * BL:(c + 1) * BL] = r["value"].T
        policy[c * BL:(c + 1) * BL, 0] = r["policy"].reshape(BL, 15, 15)
    return (value, policy), res


def kernel(board_input, params):
    (value, policy), _ = _run(board_input, params)
    return value, policy
